# revision 1
# baseline (speedup 1.0000x reference)
"""AttentiveFP readout kernel for 8 Trainium2 NeuronCores.

Strategy: graph-contiguous sharding of the V=500k nodes across 8 cores
(seg_ids sorted => each graph's nodes contiguous; split at graph
boundaries nearest V/8 multiples). Every graph lives entirely on one
core, so all segment ops are core-local and no collectives are needed.

Per core: graphs are processed in tiles of 128 (partition dim = graph).
Each graph-tile's nodes (<= NSUB*128, host-padded) are streamed through
SBUF once. Segment sum / weighted segment sum are TensorEngine matmuls
against a one-hot node->graph membership matrix built on-device via
iota==segrel compare. The attention-weighted projection uses
  g_repr = (sum_v a_v * x_v) @ Wp.T + bp   (since sum_v a_v = 1)
so the only V-sized matmuls are the K=128 one-hot reductions.
Segment softmax skips the max-subtraction (|z| <~ 12, exp is safe in
fp32). GRU runs per 128-graph tile on-chip.
"""

import numpy as np
from contextlib import ExitStack

import concourse.bass as bass
import concourse.bacc as bacc
import concourse.mybir as mybir
from concourse import tile
from concourse.bass_utils import run_bass_kernel_spmd

F32 = mybir.dt.float32
BF16 = mybir.dt.bfloat16
NP_BF16 = mybir.dt.np(mybir.dt.bfloat16)
AOP = mybir.AluOpType
ACT = mybir.ActivationFunctionType
AX = mybir.AxisListType

NCORES = 8
F = 256
T = 2
GT = 128  # graphs per tile (partition dim)
LAST_RESULT = None


def _build_program(NT_G, NSUB, bl_vals, stage=99):
    """Build the per-core SPMD program. Returns (nc, ctx).
    stage: debug cutoff; 99 = full program."""
    ctx = ExitStack()
    nc = bacc.Bacc("TRN2")
    epsc = nc.alloc_sbuf_tensor("const-f32-eps", [128, 1], F32)
    nc.gpsimd.memset(epsc.ap(), 1e-30)
    blc = []
    for t in range(T):
        bt = nc.alloc_sbuf_tensor(f"const-f32-bl{t}", [128, 1], F32)
        nc.gpsimd.memset(bt.ap(), float(bl_vals[t]))
        blc.append(bt)
    nc.all_engine_barrier()

    nf_d = nc.dram_tensor("nf", [NT_G * NSUB * 128, F], F32, kind="ExternalInput")
    segrel_d = nc.dram_tensor("segrel", [NT_G * 128, NSUB], F32, kind="ExternalInput")
    iota_d = nc.dram_tensor("iota", [128, 128], F32, kind="ExternalInput")
    identb_d = nc.dram_tensor("identb", [128, 128], BF16, kind="ExternalInput")
    identf_d = nc.dram_tensor("identf", [128, 128], F32, kind="ExternalInput")
    ones1_d = nc.dram_tensor("ones1", [1, 128], F32, kind="ExternalInput")
    wlg_d = [nc.dram_tensor(f"wlg{t}", [128, F], BF16, kind="ExternalInput") for t in range(T)]
    wln_d = [nc.dram_tensor(f"wln{t}", [128, F], BF16, kind="ExternalInput") for t in range(T)]
    wpt_d = [nc.dram_tensor(f"wpt{t}", [F, F], BF16, kind="ExternalInput") for t in range(T)]
    wih_d = [nc.dram_tensor(f"wiht{t}", [F, 3 * F], BF16, kind="ExternalInput") for t in range(T)]
    whh_d = [nc.dram_tensor(f"whht{t}", [F, 3 * F], BF16, kind="ExternalInput") for t in range(T)]
    brz_d = [nc.dram_tensor(f"brz{t}", [128, 2 * F], F32, kind="ExternalInput") for t in range(T)]
    bin_d = [nc.dram_tensor(f"bin{t}", [128, F], F32, kind="ExternalInput") for t in range(T)]
    bhn_d = [nc.dram_tensor(f"bhn{t}", [128, F], F32, kind="ExternalInput") for t in range(T)]
    bpb_d = [nc.dram_tensor(f"bpb{t}", [128, F], F32, kind="ExternalInput") for t in range(T)]
    out_d = nc.dram_tensor("out", [NT_G * 128, F], F32, kind="ExternalOutput")

    with tile.TileContext(nc) as tc:
      with tc.sbuf_pool(name="const", bufs=1) as cpool, \
           tc.sbuf_pool(name="work", bufs=2) as wpool, \
           tc.sbuf_pool(name="small", bufs=2) as spool, \
           tc.sbuf_pool(name="scr", bufs=3) as scrpool, \
           tc.sbuf_pool(name="stage", bufs=6) as stpool, \
           tc.psum_pool(name="pacc", bufs=2) as pacc, \
           tc.psum_pool(name="prz", bufs=2) as prz, \
           tc.psum_pool(name="ptiny", bufs=4) as ptiny:

        iota_sb = cpool.tile_from(iota_d[:, :], name="iota_sb")
        identb_sb = cpool.tile_from(identb_d[:, :], name="identb_sb")
        identf_sb = cpool.tile_from(identf_d[:, :], name="identf_sb")
        ones1_sb = cpool.tile_from(ones1_d[:, :], name="ones1_sb")
        wlg_sb = [cpool.tile_from(wlg_d[t][:, :], name=f"wlg_sb{t}") for t in range(T)]
        wln_sb = [cpool.tile_from(wln_d[t][:, :], name=f"wln_sb{t}") for t in range(T)]
        brz_sb = [cpool.tile_from(brz_d[t][:, :], name=f"brz_sb{t}") for t in range(T)]
        bin_sb = [cpool.tile_from(bin_d[t][:, :], name=f"bin_sb{t}") for t in range(T)]
        bhn_sb = [cpool.tile_from(bhn_d[t][:, :], name=f"bhn_sb{t}") for t in range(T)]
        bpb_sb = [cpool.tile_from(bpb_d[t][:, :], name=f"bpb_sb{t}") for t in range(T)]
        # K-chunked weights: [128, 2, N] with chunk k = rows k*128..k*128+128
        wpt_sb, wih_sb, whh_sb = [], [], []
        for t in range(T):
            wp_t = cpool.tile([128, 2, F], BF16, name=f"wp_sb{t}")
            wi_t = cpool.tile([128, 2, 3 * F], BF16, name=f"wi_sb{t}")
            wh_t = cpool.tile([128, 2, 3 * F], BF16, name=f"wh_sb{t}")
            for k in range(2):
                nc.sync.dma_start(wp_t[:, k, :], wpt_d[t][k * 128:(k + 1) * 128, :])
                nc.sync.dma_start(wi_t[:, k, :], wih_d[t][k * 128:(k + 1) * 128, :])
                nc.sync.dma_start(wh_t[:, k, :], whh_d[t][k * 128:(k + 1) * 128, :])
            wpt_sb.append(wp_t)
            wih_sb.append(wi_t)
            whh_sb.append(wh_t)

        for j in range(NT_G):
            segrel_sb = wpool.tile([128, NSUB], F32, name=f"segrel_{j}", tag="segrel")
            nc.sync.dma_start(segrel_sb[:, :], segrel_d[j * 128:(j + 1) * 128, :])
            nf_aug = wpool.tile([128, NSUB, F + 1], BF16, name=f"nfaug_{j}", tag="nfaug")
            Mn = wpool.tile([128, NSUB, 128], BF16, name=f"Mn_{j}", tag="Mn")
            nc.gpsimd.memset(nf_aug[:, :, 0], 1.0)
            ps_g0 = pacc.tile([128, F + 1], F32, name=f"psg0_{j}", tag="acc")
            for s in range(NSUB):
                stg = stpool.tile([128, F], F32, name=f"stg_{j}_{s}", tag="stage")
                r0 = (j * NSUB + s) * 128
                nc.sync.dma_start(stg[:, :], nf_d[r0:r0 + 128, :])
                if s % 2 == 0:
                    nc.vector.tensor_copy(nf_aug[:, s, 1:F + 1], stg[:, :])
                else:
                    nc.scalar.copy(nf_aug[:, s, 1:F + 1], stg[:, :])
                nc.vector.tensor_tensor(
                    Mn[:, s, :], segrel_sb[:, s:s + 1].broadcast_to((128, 128)),
                    iota_sb[:, :], op=AOP.is_equal)
                nc.tensor.matmul(ps_g0[:, 0:F], Mn[:, s, :],
                                 nf_aug[:, s, 1:F + 1],
                                 start=(s == 0), stop=(s == NSUB - 1))
            # per-node logits' node part: w[t][:, s] = nf . wl_n[t]
            w01 = wpool.tile([128, T, NSUB], F32, name=f"w01_{j}", tag="w01")
            for t in range(T):
                scrw = scrpool.tile([128, NSUB, F], BF16, name=f"scrw_{j}_{t}", tag="scr")
                nc.vector.tensor_tensor(
                    scrw[:, :, :], nf_aug[:, :, 1:F + 1],
                    wln_sb[t][:, :].unsqueeze(1).broadcast_to((128, NSUB, F)),
                    op=AOP.mult)
                nc.vector.reduce_sum(w01[:, t, :], scrw[:, :, :], axis=AX.X)
            gf = spool.tile([128, F], F32, name=f"gf0_{j}", tag="gf", bufs=6)
            nc.scalar.copy(gf[:, :], ps_g0[:, 0:F])

            if stage <= 1:
                nc.sync.dma_start(out_d[j * 128:(j + 1) * 128, :], gf[:, :])
                continue
            for t in range(T):
                # u_g = relu(gf) . wl_g   (per graph), broadcast to nodes
                rgf = spool.tile([128, F], BF16, name=f"rgf_{j}_{t}", tag="rgf")
                nc.scalar.activation(rgf[:, :], gf[:, :], ACT.Relu)
                ucol = spool.tile([128, 1], F32, name=f"ucol_{j}_{t}", tag="ucol")
                uscr = scrpool.tile([128, F], BF16, name=f"uscr_{j}_{t}", tag="uscr")
                nc.vector.tensor_tensor(uscr[:, :], rgf[:, :],
                                        wlg_sb[t][:, :], op=AOP.mult)
                nc.vector.reduce_sum(ucol[:, :], uscr[:, :], axis=AX.X)
                if stage <= 11:
                    nc.vector.tensor_copy(gf[:, 0:1], ucol[:, :])
                    continue
                urow_ps = ptiny.tile([1, 128], F32, name=f"urps_{j}_{t}", tag="tiny")
                nc.tensor.transpose(urow_ps[:, :], ucol[:, :], identf_sb[:, :])
                urow = spool.tile([1, 128], F32, name=f"urow_{j}_{t}", tag="urow")
                nc.scalar.copy(urow[:, :], urow_ps[:, :])
                if stage <= 12:
                    nc.vector.tensor_copy(gf[0:1, :], urow[:, :])
                    continue
                ubc_ps = ptiny.tile([128, 128], F32, name=f"ubcps_{j}_{t}", tag="tiny")
                nc.tensor.matmul(ubc_ps[:, :], ones1_sb[:, :], urow[:, :],
                                 start=True, stop=True)
                ubc = spool.tile([128, 128], BF16, name=f"ubc_{j}_{t}", tag="ubc")
                nc.scalar.copy(ubc[:, :], ubc_ps[:, :])
                if stage <= 13:
                    nc.vector.tensor_copy(gf[:, 0:128], ubc[:, :])
                    continue
                scr2 = scrpool.tile([128, NSUB, 128], BF16, name=f"scr2_{j}_{t}", tag="scr")
                nc.vector.tensor_tensor(
                    scr2[:, :, :], Mn[:, :, :],
                    ubc[:, :].unsqueeze(1).broadcast_to((128, NSUB, 128)),
                    op=AOP.mult)
                ubcv = spool.tile([128, NSUB], F32, name=f"ubcv_{j}_{t}", tag="ubcv")
                nc.vector.reduce_sum(ubcv[:, :], scr2[:, :, :], axis=AX.X)
                if stage <= 14:
                    nc.vector.tensor_copy(gf[:, 0:NSUB], ubcv[:, :])
                    continue
                zt0 = spool.tile([128, NSUB], F32, name=f"zt0_{j}_{t}", tag="zt0")
                nc.vector.tensor_tensor(zt0[:, :], ubcv[:, :], w01[:, t, :],
                                        op=AOP.add)
                zt = spool.tile([128, NSUB], F32, name=f"zt_{j}_{t}", tag="zt")
                nc.vector.tensor_tensor(zt[:, :], zt0[:, :],
                                        blc[t].ap().broadcast_to((128, NSUB)),
                                        op=AOP.add)
                zs = spool.tile([128, NSUB], F32, name=f"zs_{j}_{t}", tag="zs")
                nc.scalar.mul(zs[:, :], zt[:, :], 0.01)
                zl = spool.tile([128, NSUB], F32, name=f"zl_{j}_{t}", tag="zl")
                nc.vector.tensor_tensor(zl[:, :], zt[:, :], zs[:, :], op=AOP.max)
                ebf = spool.tile([128, NSUB], BF16, name=f"ebf_{j}_{t}", tag="ebf")
                nc.scalar.activation(ebf[:, :], zl[:, :], ACT.Exp)
                if stage <= 2:
                    nc.vector.tensor_copy(gf[:, 0:NSUB], ebf[:, :])
                    continue
                # weighted per-node features [e | e*x] and segment-reduce
                scr3 = scrpool.tile([128, NSUB, F + 1], BF16, name=f"scr3_{j}_{t}", tag="scr")
                nc.vector.tensor_tensor(
                    scr3[:, :, :], nf_aug[:, :, :],
                    ebf[:, :].unsqueeze(2).broadcast_to((128, NSUB, F + 1)),
                    op=AOP.mult)
                ps_ds = pacc.tile([128, F + 1], F32, name=f"psds_{j}_{t}", tag="acc")
                for s in range(NSUB):
                    nc.tensor.matmul(ps_ds[:, :], Mn[:, s, :], scr3[:, s, :],
                                     start=(s == 0), stop=(s == NSUB - 1))
                dplus = spool.tile([128, 1], F32, name=f"dplus_{j}_{t}", tag="dplus")
                nc.vector.tensor_tensor(dplus[:, :], ps_ds[:, 0:1], epsc.ap(),
                                        op=AOP.max)
                recd = spool.tile([128, 1], F32, name=f"recd_{j}_{t}", tag="recd")
                nc.vector.reciprocal(recd[:, :], dplus[:, :])
                stl = spool.tile([128, F], BF16, name=f"stl_{j}_{t}", tag="stl")
                nc.vector.tensor_tensor(stl[:, :], ps_ds[:, 1:F + 1],
                                        recd[:, :].broadcast_to((128, F)),
                                        op=AOP.mult)
                if stage <= 3:
                    nc.vector.tensor_copy(gf[:, :], stl[:, :])
                    continue
                # g_repr = stl @ Wp.T  (via transposed stl chunks)
                stT = spool.tile([128, 2, 128], BF16, name=f"stT_{j}_{t}", tag="stT")
                for k in range(2):
                    pst = ptiny.tile([128, 128], BF16, name=f"pst_{j}_{t}_{k}", tag="tiny")
                    nc.tensor.transpose(pst[:, :], stl[:, k * 128:(k + 1) * 128],
                                        identb_sb[:, :])
                    nc.scalar.copy(stT[:, k, :], pst[:, :])
                ps_wp = ptiny.tile([128, F], F32, name=f"pswp_{j}_{t}", tag="tiny")
                for k in range(2):
                    nc.tensor.matmul(ps_wp[:, :], stT[:, k, :], wpt_sb[t][:, k, :],
                                     start=(k == 0), stop=(k == 1))
                # context = elu(g_repr + bp) = relu(x) + exp(min(x,0)) - 1
                xg = spool.tile([128, F], F32, name=f"xg_{j}_{t}", tag="xg")
                nc.vector.tensor_tensor(xg[:, :], ps_wp[:, :], bpb_sb[t][:, :], op=AOP.add)
                xn = spool.tile([128, F], F32, name=f"xn_{j}_{t}", tag="xn")
                nc.vector.tensor_tensor(xn[:, :], xg[:, :],
                                        nc.const_aps.tensor(0.0, (128, F)),
                                        op=AOP.min)
                en = spool.tile([128, F], F32, name=f"en_{j}_{t}", tag="en")
                nc.scalar.activation(en[:, :], xn[:, :], ACT.Exp)
                xp = spool.tile([128, F], F32, name=f"xp_{j}_{t}", tag="xp")
                nc.scalar.activation(xp[:, :], xg[:, :], ACT.Relu)
                s1 = spool.tile([128, F], F32, name=f"s1_{j}_{t}", tag="s1")
                nc.vector.tensor_tensor(s1[:, :], en[:, :], xp[:, :], op=AOP.add)
                ctxb = spool.tile([128, F], BF16, name=f"ctxb_{j}_{t}", tag="ctxb")
                nc.vector.tensor_tensor(ctxb[:, :], s1[:, :],
                                        nc.const_aps.tensor(1.0, (128, F)),
                                        op=AOP.subtract)
                if stage <= 4:
                    nc.vector.tensor_copy(gf[:, :], ctxb[:, :])
                    continue
                # GRU(x=ctxb, h=gf)
                gfb = spool.tile([128, F], BF16, name=f"gfb_{j}_{t}", tag="gfb")
                nc.scalar.copy(gfb[:, :], gf[:, :])
                xT = spool.tile([128, 2, 128], BF16, name=f"xT_{j}_{t}", tag="xT")
                hT = spool.tile([128, 2, 128], BF16, name=f"hT_{j}_{t}", tag="hT")
                for k in range(2):
                    p1 = ptiny.tile([128, 128], BF16, name=f"p1_{j}_{t}_{k}", tag="tiny")
                    nc.tensor.transpose(p1[:, :], ctxb[:, k * 128:(k + 1) * 128],
                                        identb_sb[:, :])
                    nc.scalar.copy(xT[:, k, :], p1[:, :])
                    p2 = ptiny.tile([128, 128], BF16, name=f"p2_{j}_{t}_{k}", tag="tiny")
                    nc.tensor.transpose(p2[:, :], gfb[:, k * 128:(k + 1) * 128],
                                        identb_sb[:, :])
                    nc.scalar.copy(hT[:, k, :], p2[:, :])
                ps_rz = prz.tile([128, 2 * F], F32, name=f"psrz_{j}_{t}", tag="rz")
                mm = 0
                for lhsT, wt in ((xT, wih_sb[t]), (hT, whh_sb[t])):
                    for k in range(2):
                        nc.tensor.matmul(ps_rz[:, :], lhsT[:, k, :],
                                         wt[:, k, 0:2 * F],
                                         start=(mm == 0), stop=(mm == 3))
                        mm += 1
                ps_in = ptiny.tile([128, F], F32, name=f"psin_{j}_{t}", tag="tiny")
                for k in range(2):
                    nc.tensor.matmul(ps_in[:, :], xT[:, k, :],
                                     wih_sb[t][:, k, 2 * F:3 * F],
                                     start=(k == 0), stop=(k == 1))
                ps_hn = ptiny.tile([128, F], F32, name=f"pshn_{j}_{t}", tag="tiny")
                for k in range(2):
                    nc.tensor.matmul(ps_hn[:, :], hT[:, k, :],
                                     whh_sb[t][:, k, 2 * F:3 * F],
                                     start=(k == 0), stop=(k == 1))
                rzs = spool.tile([128, 2 * F], F32, name=f"rzs_{j}_{t}", tag="rzs")
                nc.vector.tensor_tensor(rzs[:, :], ps_rz[:, :], brz_sb[t][:, :], op=AOP.add)
                rza = spool.tile([128, 2 * F], F32, name=f"rza_{j}_{t}", tag="rza")
                nc.scalar.activation(rza[:, :], rzs[:, :], ACT.Sigmoid)
                hns = spool.tile([128, F], F32, name=f"hns_{j}_{t}", tag="hns")
                nc.vector.tensor_tensor(hns[:, :], ps_hn[:, :], bhn_sb[t][:, :], op=AOP.add)
                tmp = spool.tile([128, F], F32, name=f"tmp_{j}_{t}", tag="tmp")
                nc.vector.tensor_tensor(tmp[:, :], rza[:, 0:F], hns[:, :], op=AOP.mult)
                t2 = spool.tile([128, F], F32, name=f"t2_{j}_{t}", tag="t2")
                nc.vector.tensor_tensor(t2[:, :], tmp[:, :], ps_in[:, :], op=AOP.add)
                t3 = spool.tile([128, F], F32, name=f"t3_{j}_{t}", tag="t3")
                nc.vector.tensor_tensor(t3[:, :], t2[:, :], bin_sb[t][:, :], op=AOP.add)
                nn = spool.tile([128, F], F32, name=f"nn_{j}_{t}", tag="nn")
                nc.scalar.activation(nn[:, :], t3[:, :], ACT.Tanh)
                hm = spool.tile([128, F], F32, name=f"hm_{j}_{t}", tag="hm")
                nc.vector.tensor_tensor(hm[:, :], gf[:, :], nn[:, :], op=AOP.subtract)
                hz = spool.tile([128, F], F32, name=f"hz_{j}_{t}", tag="hz")
                nc.vector.tensor_tensor(hz[:, :], hm[:, :], rza[:, F:2 * F], op=AOP.mult)
                gf_new = spool.tile([128, F], F32, name=f"gfn_{j}_{t}", tag="gf", bufs=6)
                nc.vector.tensor_tensor(gf_new[:, :], hz[:, :], nn[:, :], op=AOP.add)
                gf = gf_new
            nc.sync.dma_start(out_d[j * 128:(j + 1) * 128, :], gf[:, :])
    nc.finalize()
    return nc, ctx


def _prep_core(node_feats, seg, g_lo, g_hi, n_lo, n_hi, NT_G, NSUB):
    """Build padded nf / segrel arrays for one core."""
    nf_pad = np.zeros((NT_G * NSUB * 128, F), np.float32)
    segrel = np.full((NT_G * 128, NSUB), -1.0, np.float32)
    for j in range(NT_G):
        gt = g_lo + j * 128
        if gt >= g_hi:
            continue
        ge = min(gt + 128, g_hi)
        a = int(np.searchsorted(seg, gt, 'left'))
        b = int(np.searchsorted(seg, ge, 'left'))
        cnt = b - a
        assert cnt <= NSUB * 128
        nf_pad[j * NSUB * 128: j * NSUB * 128 + cnt] = node_feats[a:b]
        rel = np.full(NSUB * 128, -1.0, np.float32)
        rel[:cnt] = (seg[a:b] - gt).astype(np.float32)
        # segrel[j*128 + p, s] = rel of node s*128+p
        segrel[j * 128:(j + 1) * 128, :] = rel.reshape(NSUB, 128).T
    return nf_pad, segrel


def kernel(node_feats, seg_ids, Wl, bl, Wp, bp, Wih, Whh, bih, bhh):
    node_feats = np.asarray(node_feats, np.float32)
    seg = np.asarray(seg_ids).astype(np.int64)
    Wl = np.asarray(Wl, np.float32)
    bl = np.asarray(bl, np.float32)
    Wp = np.asarray(Wp, np.float32)
    bp = np.asarray(bp, np.float32)
    Wih = np.asarray(Wih, np.float32)
    Whh = np.asarray(Whh, np.float32)
    bih = np.asarray(bih, np.float32)
    bhh = np.asarray(bhh, np.float32)
    V = node_feats.shape[0]
    G = 25000

    # graph-contiguous shard boundaries
    bounds_g = [0]
    for c in range(1, NCORES):
        bounds_g.append(int(seg[c * V // NCORES]))
    bounds_g.append(G)
    bounds_n = [int(np.searchsorted(seg, g, 'left')) for g in bounds_g]

    NT_G = max((bounds_g[c + 1] - bounds_g[c] + 127) // 128 for c in range(NCORES))
    maxnodes = 1
    for c in range(NCORES):
        for gt in range(bounds_g[c], bounds_g[c + 1], 128):
            ge = min(gt + 128, bounds_g[c + 1])
            a = np.searchsorted(seg, gt, 'left')
            b = np.searchsorted(seg, ge, 'left')
            maxnodes = max(maxnodes, int(b - a))
    NSUB = (maxnodes + 127) // 128

    nc, ctx = _build_program(NT_G, NSUB, [float(bl[t, 0]) for t in range(T)])

    # shared (replicated) weight arrays
    shared = {
        "iota": np.broadcast_to(np.arange(128, dtype=np.float32), (128, 128)).copy(),
        "identb": np.eye(128, dtype=np.float32).astype(NP_BF16),
        "identf": np.eye(128, dtype=np.float32),
        "ones1": np.ones((1, 128), np.float32),
    }
    for t in range(T):
        shared[f"wlg{t}"] = np.broadcast_to(Wl[t, 0, :F], (128, F)).astype(NP_BF16)
        shared[f"wln{t}"] = np.broadcast_to(Wl[t, 0, F:], (128, F)).astype(NP_BF16)
        shared[f"wpt{t}"] = Wp[t].T.copy().astype(NP_BF16)
        shared[f"wiht{t}"] = Wih[t].T.copy().astype(NP_BF16)
        shared[f"whht{t}"] = Whh[t].T.copy().astype(NP_BF16)
        shared[f"brz{t}"] = np.broadcast_to(bih[t, :2 * F] + bhh[t, :2 * F], (128, 2 * F)).astype(np.float32).copy()
        shared[f"bin{t}"] = np.broadcast_to(bih[t, 2 * F:], (128, F)).astype(np.float32).copy()
        shared[f"bhn{t}"] = np.broadcast_to(bhh[t, 2 * F:], (128, F)).astype(np.float32).copy()
        shared[f"bpb{t}"] = np.broadcast_to(bp[t], (128, F)).astype(np.float32).copy()

    in_maps = []
    for c in range(NCORES):
        nf_pad, segrel = _prep_core(
            node_feats, seg, bounds_g[c], bounds_g[c + 1],
            bounds_n[c], bounds_n[c + 1], NT_G, NSUB)
        m = dict(shared)
        m["nf"] = nf_pad
        m["segrel"] = segrel
        in_maps.append(m)

    res = run_bass_kernel_spmd(nc, in_maps, core_ids=list(range(NCORES)))
    ctx.close()
    global LAST_RESULT
    LAST_RESULT = res

    out = np.zeros((G, F), np.float32)
    for c in range(NCORES):
        gc = bounds_g[c + 1] - bounds_g[c]
        out[bounds_g[c]:bounds_g[c + 1]] = res.results[c]["out"][:gc]
    return out



# revision 8
# speedup vs baseline: 1.0081x; 1.0081x over previous
"""AttentiveFP readout kernel for 8 Trainium2 NeuronCores (v2).

Graph-contiguous sharding of V=500k nodes across 8 cores (seg_ids
sorted; split at graph boundaries). All segment ops core-local, no
collectives.

v2 engine plan (vs v1 which was vector-bound at 70%):
- node features staged by HOST as bf16 in four device layouts:
  nfaug  [128p, NT, NSUB, 257]  (col0 = valid flag, cols 1.. = nf)
  nft    [128f, NT, 2, NSUB*128] (transposed, for w01 on PE)
  mn     [128p, NT, NSUB, 128]  one-hot node->graph (matmul stationary)
  mnt    [128g, NT, NSUB, 128]  its transpose (u-gather on PE)
- per-node logits w01 = nf . wln_t : PE matmuls, nfT stationary, N=2.
- u broadcast/gather to nodes: PE matmuls MnT_s^T @ ucol, N=1.
- e = exp(lrelu(z)) via sigmoid identity (exp table never loaded:
  single resident ACT table set -> no ACT_TABLE_LOAD thrash):
    q = (sig(-z/4)-1)/sig(-z/4) = -e^{z/4};  e = (q^2)^2
- e*nf scaling per subtile via tensor_scalar with per-partition scalar
  (4x DVE mode), split across DVE/ACT/GPSIMD.
- elu(x) = relu(x) + min(e^x, 1) - 1, e^x via sigmoid ratio (x<=0 so
  no cancellation); the -1 is folded into the GRU input bias on host.
- GRU/Wp biases folded into K=1 ones-row matmuls.
- segment sums g0/ds: PE matmuls with Mn stationary (as v1).
"""

import numpy as np
from contextlib import ExitStack

import concourse.bass as bass
import concourse.bacc as bacc
import concourse.mybir as mybir
from concourse import tile
from concourse.bass_utils import run_bass_kernel_spmd

F32 = mybir.dt.float32
BF16 = mybir.dt.bfloat16
NP_BF16 = mybir.dt.np(mybir.dt.bfloat16)
AOP = mybir.AluOpType
ACT = mybir.ActivationFunctionType
AX = mybir.AxisListType

NCORES = 8
F = 256
T = 2
GT = 128  # graphs per tile (PSUM partition dim)
LAST_RESULT = None


def _build_program(NT, NSUB, bl_vals):
    ctx = ExitStack()
    nc = bacc.Bacc("TRN2")
    nc.all_engine_barrier()

    NN = NSUB * 128  # node slots per tile

    nfaug_d = nc.dram_tensor("nfaug", [128, NT, NSUB, F + 1], BF16, kind="ExternalInput")
    nft_d = nc.dram_tensor("nft", [128, NT, 2, NN], BF16, kind="ExternalInput")
    mn_d = nc.dram_tensor("mn", [128, NT, NSUB, 128], BF16, kind="ExternalInput")
    mnt_d = nc.dram_tensor("mnt", [128, NT, NSUB, 128], BF16, kind="ExternalInput")
    identb_d = nc.dram_tensor("identb", [128, 128], BF16, kind="ExternalInput")
    onesrow_d = nc.dram_tensor("onesrow", [1, 128], BF16, kind="ExternalInput")
    wl2_d = nc.dram_tensor("wl2", [128, 2, T], BF16, kind="ExternalInput")
    wlg_d = [nc.dram_tensor(f"wlg{t}", [128, F], BF16, kind="ExternalInput") for t in range(T)]
    wpt_d = [nc.dram_tensor(f"wpt{t}", [128, 2, F], BF16, kind="ExternalInput") for t in range(T)]
    wih_d = [nc.dram_tensor(f"wih{t}", [128, 2, 3 * F], BF16, kind="ExternalInput") for t in range(T)]
    whh_d = [nc.dram_tensor(f"whh{t}", [128, 2, 3 * F], BF16, kind="ExternalInput") for t in range(T)]
    bprow_d = [nc.dram_tensor(f"bprow{t}", [1, F], BF16, kind="ExternalInput") for t in range(T)]
    brzrow_d = [nc.dram_tensor(f"brzrow{t}", [1, 2 * F], BF16, kind="ExternalInput") for t in range(T)]
    binrow_d = [nc.dram_tensor(f"binrow{t}", [1, F], BF16, kind="ExternalInput") for t in range(T)]
    bhnrow_d = [nc.dram_tensor(f"bhnrow{t}", [1, F], BF16, kind="ExternalInput") for t in range(T)]
    out_d = nc.dram_tensor("out", [NT * 128, F], F32, kind="ExternalOutput")

    with tile.TileContext(nc) as tc:
      with tc.sbuf_pool(name="const", bufs=1) as cpool, \
           tc.sbuf_pool(name="work", bufs=2) as wpool, \
           tc.sbuf_pool(name="scr", bufs=2) as scrpool, \
           tc.sbuf_pool(name="small", bufs=3) as spool, \
           tc.psum_pool(name="pacc", bufs=3) as pacc, \
           tc.psum_pool(name="prz", bufs=2) as prz, \
           tc.psum_pool(name="ptiny", bufs=3) as ptiny:

        identb = cpool.tile_from(identb_d[:, :], name="identb")
        onesrow = cpool.tile_from(onesrow_d[:, :], name="onesrow")
        wl2 = cpool.tile_from(wl2_d[:, :, :], name="wl2")
        wlg = [cpool.tile_from(wlg_d[t][:, :], name=f"wlg{t}") for t in range(T)]
        wpt = [cpool.tile_from(wpt_d[t][:, :, :], name=f"wpt{t}") for t in range(T)]
        wih = [cpool.tile_from(wih_d[t][:, :, :], name=f"wih{t}") for t in range(T)]
        whh = [cpool.tile_from(whh_d[t][:, :, :], name=f"whh{t}") for t in range(T)]
        bprow = [cpool.tile_from(bprow_d[t][:, :], name=f"bprow{t}") for t in range(T)]
        brzrow = [cpool.tile_from(brzrow_d[t][:, :], name=f"brzrow{t}") for t in range(T)]
        binrow = [cpool.tile_from(binrow_d[t][:, :], name=f"binrow{t}") for t in range(T)]
        bhnrow = [cpool.tile_from(bhnrow_d[t][:, :], name=f"bhnrow{t}") for t in range(T)]

        for j in range(NT):
            nfaug = wpool.tile([128, NSUB, F + 1], BF16, name=f"nfaug{j}", tag="nfaug")
            nft = wpool.tile([128, 2, NN], BF16, name=f"nft{j}", tag="nft")
            mn = wpool.tile([128, NSUB, 128], BF16, name=f"mn{j}", tag="mn")
            mnt = wpool.tile([128, NSUB, 128], BF16, name=f"mnt{j}", tag="mnt")
            nc.sync.dma_start(nfaug[:, :, :], nfaug_d[:, j, :, :])
            nc.scalar.dma_start(nft[:, :, :], nft_d[:, j, :, :])
            nc.gpsimd.dma_start(mn[:, :, :], mn_d[:, j, :, :])
            nc.sync.dma_start(mnt[:, :, :], mnt_d[:, j, :, :])

            # ---- initial graph feats: g0[g,f] = sum_v Mn[v,g] nf[v,f]
            ps_g0 = pacc.tile([128, F], F32, name=f"psg0_{j}", tag="acc")
            for s in range(NSUB):
                nc.tensor.matmul(ps_g0[:, :], mn[:, s, :], nfaug[:, s, 1:F + 1],
                                 start=(s == 0), stop=(s == NSUB - 1))
            gf = spool.tile([128, F], BF16, name=f"gf0_{j}", tag="gf", bufs=4)
            nc.scalar.copy(gf[:, :], ps_g0[:, :])

            # ---- w01[v,t] = nf[v,:] . wln_t : nfT chunks stationary, N=2
            ps_w01 = ptiny.tile([128, NSUB, T], F32, name=f"psw01_{j}", tag="tiny")
            for s in range(NSUB):
                for c in range(2):
                    nc.tensor.matmul(ps_w01[:, s, :],
                                     nft[:, c, s * 128:(s + 1) * 128],
                                     wl2[:, c, :],
                                     start=(c == 0), stop=(c == 1))
            # -> [128, T, NSUB] bf16 (t-major so later slices are packed)
            w01sb = spool.tile([128, T, NSUB], BF16, name=f"w01_{j}", tag="w01")
            nc.vector.tensor_copy(
                w01sb[:, :, :],
                ps_w01[:, :, :].transpose((0, 2, 1)))

            for t in range(T):
                # ---- u_g = relu(gf).wlg_t (fused STT with accumulate)
                uscr = spool.tile([128, F], BF16, name=f"uscr_{j}_{t}", tag="uscr")
                ucol = spool.tile([128, 1], F32, name=f"ucol_{j}_{t}", tag="ucol")
                nc.vector.scalar_tensor_tensor(
                    uscr[:, :], gf[:, :], 0.0, wlg[t][:, :],
                    op0=AOP.max, op1=AOP.mult, accum_out=ucol[:, :])
                # ucol + bl_t, cast bf16 (matmul rhs)
                ucolb = spool.tile([128, 1], BF16, name=f"ucolb_{j}_{t}", tag="ucolb")
                nc.scalar.activation(ucolb[:, :], ucol[:, :], ACT.Copy,
                                     bias=float(bl_vals[t]))
                # ---- gather u to nodes: ubcv[v] = sum_g MnT[g,v] u[g]
                ps_ubcv = ptiny.tile([128, NSUB], F32, name=f"psub_{j}_{t}", tag="tiny")
                for s in range(NSUB):
                    nc.tensor.matmul(ps_ubcv[:, s:s + 1], mnt[:, s, :],
                                     ucolb[:, :], start=True, stop=True)
                # ---- z = lrelu(w01 + u + bl); e = exp(z) via sigmoid:
                #   sm = sig(-z/4); q = (sm-1)/sm = -e^{z/4}; e = (q^2)^2
                zt = spool.tile([128, NSUB], F32, name=f"zt_{j}_{t}", tag="zt")
                nc.vector.tensor_tensor(zt[:, :], ps_ubcv[:, :], w01sb[:, t, :],
                                        op=AOP.add)
                zl = spool.tile([128, NSUB], F32, name=f"zl_{j}_{t}", tag="zl")
                nc.scalar.activation(zl[:, :], zt[:, :], ACT.Prelu, alpha=0.01)
                sm = spool.tile([128, NSUB], F32, name=f"sm_{j}_{t}", tag="sm")
                nc.scalar.activation(sm[:, :], zl[:, :], ACT.Sigmoid, scale=-0.25)
                rsm = spool.tile([128, NSUB], F32, name=f"rsm_{j}_{t}", tag="rsm")
                nc.vector.reciprocal(rsm[:, :], sm[:, :])
                q4 = spool.tile([128, NSUB], F32, name=f"q4_{j}_{t}", tag="q4")
                nc.vector.tensor_scalar(q4[:, :], rsm[:, :], -1.0, 1.0,
                                        op0=AOP.mult, op1=AOP.add)
                q2 = spool.tile([128, NSUB], F32, name=f"q2_{j}_{t}", tag="q2")
                nc.vector.tensor_tensor(q2[:, :], q4[:, :], q4[:, :], op=AOP.mult)
                ebf = spool.tile([128, NSUB], F32, name=f"ebf_{j}_{t}", tag="ebf")
                nc.vector.tensor_tensor(ebf[:, :], q2[:, :], q2[:, :], op=AOP.mult)

                # ---- scr3 = e * [valid | nf] per subtile (TS, per-partition
                # scalar keeps 4x mode); split across DVE/ACT/GPSIMD
                scr3 = scrpool.tile([128, NSUB, F + 1], BF16, name=f"scr3_{j}_{t}", tag="scr3")
                for s in range(NSUB):
                    r = s % 4
                    if r in (0, 1):
                        nc.vector.tensor_scalar(
                            scr3[:, s, :], nfaug[:, s, :], ebf[:, s:s + 1], None,
                            op0=AOP.mult)
                    elif r == 2:
                        nc.scalar.activation(
                            scr3[:, s, :], nfaug[:, s, :], ACT.Copy,
                            scale=ebf[:, s:s + 1])
                    else:
                        nc.gpsimd.tensor_scalar(
                            scr3[:, s, :], nfaug[:, s, :], ebf[:, s:s + 1], None,
                            op0=AOP.mult)
                # ---- ds[g, 0:257] = sum_v Mn[v,g] scr3[v,:]
                ps_ds = pacc.tile([128, F + 1], F32, name=f"psds_{j}_{t}", tag="acc")
                for s in range(NSUB):
                    nc.tensor.matmul(ps_ds[:, :], mn[:, s, :], scr3[:, s, :],
                                     start=(s == 0), stop=(s == NSUB - 1))
                # ---- stl = ds[:,1:]/max(ds[:,0], eps)
                dplus = spool.tile([128, 1], F32, name=f"dp_{j}_{t}", tag="dp")
                nc.vector.tensor_scalar(dplus[:, :], ps_ds[:, 0:1], 1e-30, None,
                                        op0=AOP.max)
                recd = spool.tile([128, 1], F32, name=f"recd_{j}_{t}", tag="recd")
                nc.vector.reciprocal(recd[:, :], dplus[:, :])
                stl = spool.tile([128, F], BF16, name=f"stl_{j}_{t}", tag="stl")
                nc.scalar.activation(stl[:, :], ps_ds[:, 1:F + 1], ACT.Copy,
                                     scale=recd[:, :])
                # ---- g_repr = stl @ Wp^T + bp  (transpose stl, then matmul)
                ps_st = ptiny.tile([128, 2, 128], BF16, name=f"psst_{j}_{t}", tag="tiny")
                for c in range(2):
                    nc.tensor.transpose(ps_st[:, c, :], stl[:, c * 128:(c + 1) * 128],
                                        identb[:, :])
                stT = spool.tile([128, 2, 128], BF16, name=f"stT_{j}_{t}", tag="stT")
                nc.vector.tensor_copy(stT[:, :, :], ps_st[:, :, :])
                ps_ctx = pacc.tile([128, F], F32, name=f"psctx_{j}_{t}", tag="acc")
                nc.tensor.matmul(ps_ctx[:, :], stT[:, 0, :], wpt[t][:, 0, :],
                                 start=True, stop=False)
                nc.tensor.matmul(ps_ctx[:, :], stT[:, 1, :], wpt[t][:, 1, :],
                                 start=False, stop=False)
                nc.tensor.matmul(ps_ctx[:, :], onesrow[:, :], bprow[t][:, :],
                                 start=False, stop=True)
                # ---- ctx+1 = relu(x) + min(e^x, 1); e^x = sg/(1-sg)
                sg = spool.tile([128, F], BF16, name=f"sg_{j}_{t}", tag="sg")
                nc.scalar.activation(sg[:, :], ps_ctx[:, :], ACT.Sigmoid)
                omy = spool.tile([128, F], F32, name=f"omy_{j}_{t}", tag="omy")
                nc.vector.tensor_scalar(omy[:, :], sg[:, :], -1.0, 1.0,
                                        op0=AOP.mult, op1=AOP.add)
                romy = spool.tile([128, F], F32, name=f"romy_{j}_{t}", tag="romy")
                nc.vector.reciprocal(romy[:, :], omy[:, :])
                exn = spool.tile([128, F], BF16, name=f"exn_{j}_{t}", tag="exn")
                nc.vector.tensor_tensor(exn[:, :], sg[:, :], romy[:, :],
                                        op=AOP.mult)
                exm = spool.tile([128, F], BF16, name=f"exm_{j}_{t}", tag="exm")
                nc.vector.tensor_scalar(exm[:, :], exn[:, :], 1.0, None,
                                        op0=AOP.min)
                ctxb = spool.tile([128, F], BF16, name=f"ctxb_{j}_{t}", tag="ctxb")
                nc.vector.scalar_tensor_tensor(
                    ctxb[:, :], ps_ctx[:, :], 0.0, exm[:, :],
                    op0=AOP.max, op1=AOP.add)
                # (the -1 of elu is folded into brzrow/binrow on host)

                # ---- GRU: transposes of x=ctxb and h=gf
                ps_tr = ptiny.tile([128, 4, 128], BF16, name=f"pstr_{j}_{t}", tag="tiny")
                for c in range(2):
                    nc.tensor.transpose(ps_tr[:, c, :], ctxb[:, c * 128:(c + 1) * 128],
                                        identb[:, :])
                    nc.tensor.transpose(ps_tr[:, 2 + c, :], gf[:, c * 128:(c + 1) * 128],
                                        identb[:, :])
                xh = spool.tile([128, 4, 128], BF16, name=f"xh_{j}_{t}", tag="xh")
                nc.vector.tensor_copy(xh[:, :, :], ps_tr[:, :, :])
                ps_rz = prz.tile([128, 2 * F], F32, name=f"psrz_{j}_{t}", tag="rz")
                mmi = 0
                for base, wt in ((0, wih[t]), (2, whh[t])):
                    for c in range(2):
                        nc.tensor.matmul(ps_rz[:, :], xh[:, base + c, :],
                                         wt[:, c, 0:2 * F],
                                         start=(mmi == 0), stop=False)
                        mmi += 1
                nc.tensor.matmul(ps_rz[:, :], onesrow[:, :], brzrow[t][:, :],
                                 start=False, stop=True)
                ps_in = pacc.tile([128, F], F32, name=f"psin_{j}_{t}", tag="acc")
                for c in range(2):
                    nc.tensor.matmul(ps_in[:, :], xh[:, c, :],
                                     wih[t][:, c, 2 * F:3 * F],
                                     start=(c == 0), stop=False)
                nc.tensor.matmul(ps_in[:, :], onesrow[:, :], binrow[t][:, :],
                                 start=False, stop=True)
                ps_hn = pacc.tile([128, F], F32, name=f"pshn_{j}_{t}", tag="acc")
                for c in range(2):
                    nc.tensor.matmul(ps_hn[:, :], xh[:, 2 + c, :],
                                     whh[t][:, c, 2 * F:3 * F],
                                     start=(c == 0), stop=False)
                nc.tensor.matmul(ps_hn[:, :], onesrow[:, :], bhnrow[t][:, :],
                                 start=False, stop=True)
                rza = spool.tile([128, 2 * F], BF16, name=f"rza_{j}_{t}", tag="rza")
                nc.scalar.activation(rza[:, :], ps_rz[:, :], ACT.Sigmoid)
                tmp = spool.tile([128, F], BF16, name=f"tmp_{j}_{t}", tag="tmp")
                nc.vector.tensor_tensor(tmp[:, :], ps_hn[:, :], rza[:, 0:F],
                                        op=AOP.mult)
                t2 = spool.tile([128, F], F32, name=f"t2_{j}_{t}", tag="t2")
                nc.vector.tensor_tensor(t2[:, :], ps_in[:, :], tmp[:, :],
                                        op=AOP.add)
                nn = spool.tile([128, F], BF16, name=f"nn_{j}_{t}", tag="nn")
                nc.scalar.activation(nn[:, :], t2[:, :], ACT.Tanh)
                hm = spool.tile([128, F], BF16, name=f"hm_{j}_{t}", tag="hm")
                nc.vector.tensor_tensor(hm[:, :], gf[:, :], nn[:, :],
                                        op=AOP.subtract)
                hz = spool.tile([128, F], BF16, name=f"hz_{j}_{t}", tag="hz")
                nc.vector.tensor_tensor(hz[:, :], hm[:, :], rza[:, F:2 * F],
                                        op=AOP.mult)
                if t < T - 1:
                    gf_new = spool.tile([128, F], BF16, name=f"gfn_{j}_{t}", tag="gf", bufs=4)
                    nc.vector.tensor_tensor(gf_new[:, :], hz[:, :], nn[:, :],
                                            op=AOP.add)
                    gf = gf_new
                else:
                    gout = spool.tile([128, F], F32, name=f"gout_{j}", tag="gout")
                    nc.vector.tensor_tensor(gout[:, :], hz[:, :], nn[:, :],
                                            op=AOP.add)
                    nc.scalar.dma_start(out_d[j * 128:(j + 1) * 128, :], gout[:, :])
    nc.finalize()
    return nc, ctx


def _prep_core(node_feats_bf, seg, g_lo, g_hi, NT, NSUB):
    """Stage one core's node data in the four device layouts."""
    NN = NSUB * 128
    nfaug = np.zeros((128, NT, NSUB, F + 1), NP_BF16)
    nft = np.zeros((128, NT, 2, NN), NP_BF16)
    mn = np.zeros((128, NT, NSUB, 128), NP_BF16)
    mnt = np.zeros((128, NT, NSUB, 128), NP_BF16)
    eye = np.eye(128, dtype=NP_BF16)
    for j in range(NT):
        gt = g_lo + j * 128
        if gt >= g_hi:
            continue
        ge = min(gt + 128, g_hi)
        a = int(np.searchsorted(seg, gt, 'left'))
        b = int(np.searchsorted(seg, ge, 'left'))
        cnt = b - a
        assert cnt <= NN
        tmp = np.zeros((NN, F + 1), NP_BF16)
        tmp[:cnt, 0] = 1.0
        tmp[:cnt, 1:] = node_feats_bf[a:b]
        # node n -> subtile s=n//128, partition p=n%128
        nfaug[:, j] = tmp.reshape(NSUB, 128, F + 1).transpose(1, 0, 2)
        # nft[fp, j, c, n] = nf[a+n, c*128+fp]
        nft[:, j] = np.ascontiguousarray(
            tmp[:, 1:].T.reshape(2, 128, NN).transpose(1, 0, 2))
        grel = np.full(NN, -1, np.int64)
        grel[:cnt] = seg[a:b] - gt
        oh = eye[np.clip(grel, 0, 127)] * (grel >= 0)[:, None].astype(NP_BF16)
        oh = oh.reshape(NSUB, 128, 128)          # [s, p, g]
        mn[:, j] = oh.transpose(1, 0, 2)         # [p, s, g]
        mnt[:, j] = oh.transpose(2, 0, 1)        # [g, s, p]
    return nfaug, nft, mn, mnt


def kernel(node_feats, seg_ids, Wl, bl, Wp, bp, Wih, Whh, bih, bhh):
    node_feats = np.asarray(node_feats, np.float32)
    seg = np.asarray(seg_ids).astype(np.int64)
    Wl = np.asarray(Wl, np.float32)
    bl = np.asarray(bl, np.float32)
    Wp = np.asarray(Wp, np.float32)
    bp = np.asarray(bp, np.float32)
    Wih = np.asarray(Wih, np.float32)
    Whh = np.asarray(Whh, np.float32)
    bih = np.asarray(bih, np.float32)
    bhh = np.asarray(bhh, np.float32)
    V = node_feats.shape[0]
    G = 25000

    bounds_g = [0]
    for c in range(1, NCORES):
        bounds_g.append(int(seg[c * V // NCORES]))
    bounds_g.append(G)

    NT = max((bounds_g[c + 1] - bounds_g[c] + 127) // 128 for c in range(NCORES))
    maxnodes = 1
    for c in range(NCORES):
        for gt in range(bounds_g[c], bounds_g[c + 1], 128):
            ge = min(gt + 128, bounds_g[c + 1])
            a = np.searchsorted(seg, gt, 'left')
            b = np.searchsorted(seg, ge, 'left')
            maxnodes = max(maxnodes, int(b - a))
    NSUB = (maxnodes + 127) // 128

    nc, ctx = _build_program(NT, NSUB, [float(bl[t, 0]) for t in range(T)])

    # ---- shared (replicated) weights
    shared = {
        "identb": np.eye(128, dtype=NP_BF16),
        "onesrow": np.ones((1, 128), NP_BF16),
    }
    # wl2[fp, c, t] = Wl[t, 0, 256 + c*128 + fp]
    wl2 = np.zeros((128, 2, T), np.float32)
    for t in range(T):
        for c in range(2):
            wl2[:, c, t] = Wl[t, 0, F + c * 128:F + (c + 1) * 128]
    shared["wl2"] = wl2.astype(NP_BF16)
    for t in range(T):
        shared[f"wlg{t}"] = np.broadcast_to(Wl[t, 0, :F], (128, F)).astype(NP_BF16)
        # wpt[fp, c, fo] = Wp[t, fo, c*128+fp]
        shared[f"wpt{t}"] = np.ascontiguousarray(
            Wp[t].T.reshape(2, 128, F).transpose(1, 0, 2)).astype(NP_BF16)
        shared[f"wih{t}"] = np.ascontiguousarray(
            Wih[t].T.reshape(2, 128, 3 * F).transpose(1, 0, 2)).astype(NP_BF16)
        shared[f"whh{t}"] = np.ascontiguousarray(
            Whh[t].T.reshape(2, 128, 3 * F).transpose(1, 0, 2)).astype(NP_BF16)
        shared[f"bprow{t}"] = bp[t][None, :].astype(NP_BF16)
        # elu's -1 shifted into the GRU input bias: x_gru = ctx+1 staged,
        # so bias_x -= rowsum(Wih)
        rs = Wih[t].sum(axis=1)
        shared[f"brzrow{t}"] = (bih[t, :2 * F] + bhh[t, :2 * F] - rs[:2 * F])[None, :].astype(NP_BF16)
        shared[f"binrow{t}"] = (bih[t, 2 * F:] - rs[2 * F:])[None, :].astype(NP_BF16)
        shared[f"bhnrow{t}"] = bhh[t, 2 * F:][None, :].astype(NP_BF16)

    node_feats_bf = node_feats.astype(NP_BF16)
    in_maps = []
    for c in range(NCORES):
        nfaug, nft, mn, mnt = _prep_core(
            node_feats_bf, seg, bounds_g[c], bounds_g[c + 1], NT, NSUB)
        m = dict(shared)
        m["nfaug"] = nfaug
        m["nft"] = nft
        m["mn"] = mn
        m["mnt"] = mnt
        in_maps.append(m)

    res = run_bass_kernel_spmd(nc, in_maps, core_ids=list(range(NCORES)))
    ctx.close()
    global LAST_RESULT
    LAST_RESULT = res

    out = np.zeros((G, F), np.float32)
    for c in range(NCORES):
        gc = bounds_g[c + 1] - bounds_g[c]
        out[bounds_g[c]:bounds_g[c + 1]] = res.results[c]["out"][:gc]
    return out


# revision 18
# speedup vs baseline: 1.1318x; 1.1227x over previous
"""AttentiveFP readout kernel for 8 Trainium2 NeuronCores (v2).

Graph-contiguous sharding of V=500k nodes across 8 cores (seg_ids
sorted; split at graph boundaries). All segment ops core-local, no
collectives.

v2 engine plan (vs v1 which was vector-bound at 70%):
- node features staged by HOST as bf16 in four device layouts:
  nfaug  [128p, NT, NSUB, 257]  (col0 = valid flag, cols 1.. = nf)
  nft    [128f, NT, 2, NSUB*128] (transposed, for w01 on PE)
  mn     [128p, NT, NSUB, 128]  one-hot node->graph (matmul stationary)
  mnt    [128g, NT, NSUB, 128]  its transpose (u-gather on PE)
- per-node logits w01 = nf . wln_t : PE matmuls, nfT stationary, N=2.
- u broadcast/gather to nodes: PE matmuls MnT_s^T @ ucol, N=1.
- e = exp(lrelu(z)) via sigmoid identity (exp table never loaded:
  single resident ACT table set -> no ACT_TABLE_LOAD thrash):
    q = (sig(-z/4)-1)/sig(-z/4) = -e^{z/4};  e = (q^2)^2
- e*nf scaling per subtile via tensor_scalar with per-partition scalar
  (4x DVE mode), split across DVE/ACT/GPSIMD.
- elu(x) = relu(x) + min(e^x, 1) - 1, e^x via sigmoid ratio (x<=0 so
  no cancellation); the -1 is folded into the GRU input bias on host.
- GRU/Wp biases folded into K=1 ones-row matmuls.
- segment sums g0/ds: PE matmuls with Mn stationary (as v1).
"""

import numpy as np
from contextlib import ExitStack

import concourse.bass as bass
import concourse.bacc as bacc
import concourse.mybir as mybir
from concourse import tile
from concourse.bass_utils import run_bass_kernel_spmd

F32 = mybir.dt.float32
BF16 = mybir.dt.bfloat16
NP_BF16 = mybir.dt.np(mybir.dt.bfloat16)
AOP = mybir.AluOpType
ACT = mybir.ActivationFunctionType
AX = mybir.AxisListType

NCORES = 8
F = 256
T = 2
GT = 128  # graphs per tile (PSUM partition dim)
LAST_RESULT = None


def _build_program(NT, NSUB, bl_vals):
    ctx = ExitStack()
    nc = bacc.Bacc("TRN2")
    nc.all_engine_barrier()

    NN = NSUB * 128  # node slots per tile

    nfaug_d = nc.dram_tensor("nfaug", [128, NT, NSUB, F + 1], BF16, kind="ExternalInput")
    nft_d = nc.dram_tensor("nft", [128, NT, 2, NN], BF16, kind="ExternalInput")
    # mn is staged g-major [p, g, s] so the e-broadcast TT keeps its
    # innermost axis packed (2x DVE mode)
    mn_d = nc.dram_tensor("mn", [128, NT, 128, NSUB], BF16, kind="ExternalInput")
    mnt_d = nc.dram_tensor("mnt", [128, NT, NSUB, 128], BF16, kind="ExternalInput")
    identb_d = nc.dram_tensor("identb", [128, 128], BF16, kind="ExternalInput")
    onesrow_d = nc.dram_tensor("onesrow", [1, 128], BF16, kind="ExternalInput")
    wl2_d = nc.dram_tensor("wl2", [128, 2, T], BF16, kind="ExternalInput")
    wlg_d = [nc.dram_tensor(f"wlg{t}", [128, F], BF16, kind="ExternalInput") for t in range(T)]
    wpt_d = [nc.dram_tensor(f"wpt{t}", [128, 2, F], BF16, kind="ExternalInput") for t in range(T)]
    wih_d = [nc.dram_tensor(f"wih{t}", [128, 2, 3 * F], BF16, kind="ExternalInput") for t in range(T)]
    whh_d = [nc.dram_tensor(f"whh{t}", [128, 2, 3 * F], BF16, kind="ExternalInput") for t in range(T)]
    bprow_d = [nc.dram_tensor(f"bprow{t}", [1, F], BF16, kind="ExternalInput") for t in range(T)]
    brzrow_d = [nc.dram_tensor(f"brzrow{t}", [1, 2 * F], BF16, kind="ExternalInput") for t in range(T)]
    binrow_d = [nc.dram_tensor(f"binrow{t}", [1, F], BF16, kind="ExternalInput") for t in range(T)]
    bhnrow_d = [nc.dram_tensor(f"bhnrow{t}", [1, F], BF16, kind="ExternalInput") for t in range(T)]
    out_d = nc.dram_tensor("out", [NT * 128, F], F32, kind="ExternalOutput")

    with tile.TileContext(nc) as tc:
      with tc.sbuf_pool(name="const", bufs=1) as cpool, \
           tc.sbuf_pool(name="work", bufs=2) as wpool, \
           tc.sbuf_pool(name="scr", bufs=2) as scrpool, \
           tc.sbuf_pool(name="small", bufs=3) as spool, \
           tc.psum_pool(name="pacc", bufs=3) as pacc, \
           tc.psum_pool(name="prz", bufs=2) as prz, \
           tc.psum_pool(name="ptiny", bufs=3) as ptiny:

        identb = cpool.tile_from(identb_d[:, :], name="identb")
        onesrow = cpool.tile_from(onesrow_d[:, :], name="onesrow")
        wl2 = cpool.tile_from(wl2_d[:, :, :], name="wl2")
        wlg = [cpool.tile_from(wlg_d[t][:, :], name=f"wlg{t}") for t in range(T)]
        wpt = [cpool.tile_from(wpt_d[t][:, :, :], name=f"wpt{t}") for t in range(T)]
        wih = [cpool.tile_from(wih_d[t][:, :, :], name=f"wih{t}") for t in range(T)]
        whh = [cpool.tile_from(whh_d[t][:, :, :], name=f"whh{t}") for t in range(T)]
        bprow = [cpool.tile_from(bprow_d[t][:, :], name=f"bprow{t}") for t in range(T)]
        brzrow = [cpool.tile_from(brzrow_d[t][:, :], name=f"brzrow{t}") for t in range(T)]
        binrow = [cpool.tile_from(binrow_d[t][:, :], name=f"binrow{t}") for t in range(T)]
        bhnrow = [cpool.tile_from(bhnrow_d[t][:, :], name=f"bhnrow{t}") for t in range(T)]

        for j in range(NT):
            nfaug = wpool.tile([128, NSUB, F + 1], BF16, name=f"nfaug{j}", tag="nfaug")
            nft = wpool.tile([128, 2, NN], BF16, name=f"nft{j}", tag="nft")
            mn = wpool.tile([128, 128, NSUB], BF16, name=f"mn{j}", tag="mn")
            mnt = wpool.tile([128, NSUB, 128], BF16, name=f"mnt{j}", tag="mnt")
            nc.sync.dma_start(nfaug[:, :, :], nfaug_d[:, j, :, :])
            nc.scalar.dma_start(nft[:, :, :], nft_d[:, j, :, :])
            nc.gpsimd.dma_start(mn[:, :, :], mn_d[:, j, :, :])
            nc.sync.dma_start(mnt[:, :, :], mnt_d[:, j, :, :])

            # ---- initial graph feats: g0[g,f] = sum_v Mn[v,g] nf[v,f]
            ps_g0 = pacc.tile([128, F], F32, name=f"psg0_{j}", tag="acc")
            for s in range(NSUB):
                nc.tensor.matmul(ps_g0[:, :], mn[:, :, s], nfaug[:, s, 1:F + 1],
                                 start=(s == 0), stop=(s == NSUB - 1))
            gf = spool.tile([128, F], BF16, name=f"gf0_{j}", tag="gf", bufs=4)
            nc.scalar.copy(gf[:, :], ps_g0[:, :])

            # ---- w01[v,t] = nf[v,:] . wln_t : nfT chunks stationary, N=2
            ps_w01 = ptiny.tile([128, NSUB, T], F32, name=f"psw01_{j}", tag="tiny")
            for s in range(NSUB):
                for c in range(2):
                    nc.tensor.matmul(ps_w01[:, s, :],
                                     nft[:, c, s * 128:(s + 1) * 128],
                                     wl2[:, c, :],
                                     start=(c == 0), stop=(c == 1))
            # -> [128, T, NSUB] bf16 (t-major so later slices are packed)
            w01sb = spool.tile([128, T, NSUB], BF16, name=f"w01_{j}", tag="w01")
            nc.vector.tensor_copy(
                w01sb[:, :, :],
                ps_w01[:, :, :].transpose((0, 2, 1)))

            for t in range(T):
                # ---- u_g = relu(gf).wlg_t (fused STT with accumulate)
                uscr = spool.tile([128, F], BF16, name=f"uscr_{j}_{t}", tag="uscr")
                ucol = spool.tile([128, 1], F32, name=f"ucol_{j}_{t}", tag="ucol")
                nc.vector.scalar_tensor_tensor(
                    uscr[:, :], gf[:, :], 0.0, wlg[t][:, :],
                    op0=AOP.max, op1=AOP.mult, accum_out=ucol[:, :])
                # ucol + bl_t, cast bf16 (matmul rhs)
                ucolb = spool.tile([128, 1], BF16, name=f"ucolb_{j}_{t}", tag="ucolb")
                nc.scalar.activation(ucolb[:, :], ucol[:, :], ACT.Copy,
                                     bias=float(bl_vals[t]))
                # ---- gather u to nodes: ubcv[v] = sum_g MnT[g,v] u[g]
                ps_ubcv = ptiny.tile([128, NSUB], F32, name=f"psub_{j}_{t}", tag="tiny")
                for s in range(NSUB):
                    nc.tensor.matmul(ps_ubcv[:, s:s + 1], mnt[:, s, :],
                                     ucolb[:, :], start=True, stop=True)
                # ---- z = lrelu(w01 + u + bl); e = exp(z) via sigmoid:
                #   sm = sig(-z/4); q = (sm-1)/sm = -e^{z/4}; e = (q^2)^2
                zt = spool.tile([128, NSUB], F32, name=f"zt_{j}_{t}", tag="zt")
                nc.vector.tensor_tensor(zt[:, :], ps_ubcv[:, :], w01sb[:, t, :],
                                        op=AOP.add)
                zl = spool.tile([128, NSUB], F32, name=f"zl_{j}_{t}", tag="zl")
                nc.scalar.activation(zl[:, :], zt[:, :], ACT.Prelu, alpha=0.01)
                sm = spool.tile([128, NSUB], F32, name=f"sm_{j}_{t}", tag="sm")
                nc.scalar.activation(sm[:, :], zl[:, :], ACT.Sigmoid, scale=-0.25)
                rsm = spool.tile([128, NSUB], F32, name=f"rsm_{j}_{t}", tag="rsm")
                nc.vector.reciprocal(rsm[:, :], sm[:, :])
                q4 = spool.tile([128, NSUB], F32, name=f"q4_{j}_{t}", tag="q4")
                nc.vector.tensor_scalar(q4[:, :], rsm[:, :], -1.0, 1.0,
                                        op0=AOP.mult, op1=AOP.add)
                q2 = spool.tile([128, NSUB], F32, name=f"q2_{j}_{t}", tag="q2")
                nc.vector.tensor_tensor(q2[:, :], q4[:, :], q4[:, :], op=AOP.mult)
                ebf = spool.tile([128, NSUB], BF16, name=f"ebf_{j}_{t}", tag="ebf")
                nc.vector.tensor_tensor(ebf[:, :], q2[:, :], q2[:, :], op=AOP.mult)

                # ---- fold e into the one-hot: Mne[v,g,s] = Mn[v,g,s]*e[v,s]
                # (packed innermost axis on all operands -> 2x DVE mode)
                mne = scrpool.tile([128, 128, NSUB], BF16, name=f"mne_{j}_{t}", tag="mne")
                nc.vector.tensor_tensor(
                    mne[:, :, :], mn[:, :, :],
                    ebf[:, :].unsqueeze(1).broadcast_to((128, 128, NSUB)),
                    op=AOP.mult)
                # ---- ds[g, 0:257] = sum_v Mne[v,g] [valid|nf][v,:]
                ps_ds = pacc.tile([128, F + 1], F32, name=f"psds_{j}_{t}", tag="acc")
                for s in range(NSUB):
                    nc.tensor.matmul(ps_ds[:, :], mne[:, :, s], nfaug[:, s, :],
                                     start=(s == 0), stop=(s == NSUB - 1))
                # ---- stl = ds[:,1:]/max(ds[:,0], eps)
                dplus = spool.tile([128, 1], F32, name=f"dp_{j}_{t}", tag="dp")
                nc.vector.tensor_scalar(dplus[:, :], ps_ds[:, 0:1], 1e-30, None,
                                        op0=AOP.max)
                recd = spool.tile([128, 1], F32, name=f"recd_{j}_{t}", tag="recd")
                nc.vector.reciprocal(recd[:, :], dplus[:, :])
                stl = spool.tile([128, F], BF16, name=f"stl_{j}_{t}", tag="stl")
                nc.scalar.activation(stl[:, :], ps_ds[:, 1:F + 1], ACT.Copy,
                                     scale=recd[:, :])
                # ---- g_repr = stl @ Wp^T + bp  (transpose stl, then matmul)
                ps_st = ptiny.tile([128, 2, 128], BF16, name=f"psst_{j}_{t}", tag="tiny")
                for c in range(2):
                    nc.tensor.transpose(ps_st[:, c, :], stl[:, c * 128:(c + 1) * 128],
                                        identb[:, :])
                stT = spool.tile([128, 2, 128], BF16, name=f"stT_{j}_{t}", tag="stT")
                nc.vector.tensor_copy(stT[:, :, :], ps_st[:, :, :])
                ps_ctx = pacc.tile([128, F], F32, name=f"psctx_{j}_{t}", tag="acc")
                nc.tensor.matmul(ps_ctx[:, :], stT[:, 0, :], wpt[t][:, 0, :],
                                 start=True, stop=False)
                nc.tensor.matmul(ps_ctx[:, :], stT[:, 1, :], wpt[t][:, 1, :],
                                 start=False, stop=False)
                nc.tensor.matmul(ps_ctx[:, :], onesrow[:, :], bprow[t][:, :],
                                 start=False, stop=True)
                # ---- ctx+1 = relu(x) + min(e^x, 1); e^x = sg/(1-sg)
                sg = spool.tile([128, F], BF16, name=f"sg_{j}_{t}", tag="sg")
                nc.scalar.activation(sg[:, :], ps_ctx[:, :], ACT.Sigmoid)
                omy = spool.tile([128, F], F32, name=f"omy_{j}_{t}", tag="omy")
                nc.gpsimd.tensor_scalar(omy[:, :], sg[:, :], -1.0, 1.0,
                                        op0=AOP.mult, op1=AOP.add)
                romy = spool.tile([128, F], F32, name=f"romy_{j}_{t}", tag="romy")
                nc.vector.reciprocal(romy[:, :], omy[:, :])
                exn = spool.tile([128, F], BF16, name=f"exn_{j}_{t}", tag="exn")
                nc.vector.tensor_tensor(exn[:, :], sg[:, :], romy[:, :],
                                        op=AOP.mult)
                exm = spool.tile([128, F], BF16, name=f"exm_{j}_{t}", tag="exm")
                nc.gpsimd.tensor_scalar(exm[:, :], exn[:, :], 1.0, None,
                                        op0=AOP.min)
                ctxb = spool.tile([128, F], BF16, name=f"ctxb_{j}_{t}", tag="ctxb")
                nc.vector.scalar_tensor_tensor(
                    ctxb[:, :], ps_ctx[:, :], 0.0, exm[:, :],
                    op0=AOP.max, op1=AOP.add)
                # (the -1 of elu is folded into brzrow/binrow on host)

                # ---- GRU: transposes of x=ctxb and h=gf
                ps_tr = ptiny.tile([128, 4, 128], BF16, name=f"pstr_{j}_{t}", tag="tiny")
                for c in range(2):
                    nc.tensor.transpose(ps_tr[:, c, :], ctxb[:, c * 128:(c + 1) * 128],
                                        identb[:, :])
                    nc.tensor.transpose(ps_tr[:, 2 + c, :], gf[:, c * 128:(c + 1) * 128],
                                        identb[:, :])
                xh = spool.tile([128, 4, 128], BF16, name=f"xh_{j}_{t}", tag="xh")
                nc.vector.tensor_copy(xh[:, :, :], ps_tr[:, :, :])
                ps_rz = prz.tile([128, 2 * F], F32, name=f"psrz_{j}_{t}", tag="rz")
                mmi = 0
                for base, wt in ((0, wih[t]), (2, whh[t])):
                    for c in range(2):
                        nc.tensor.matmul(ps_rz[:, :], xh[:, base + c, :],
                                         wt[:, c, 0:2 * F],
                                         start=(mmi == 0), stop=False)
                        mmi += 1
                nc.tensor.matmul(ps_rz[:, :], onesrow[:, :], brzrow[t][:, :],
                                 start=False, stop=True)
                ps_in = pacc.tile([128, F], F32, name=f"psin_{j}_{t}", tag="acc")
                for c in range(2):
                    nc.tensor.matmul(ps_in[:, :], xh[:, c, :],
                                     wih[t][:, c, 2 * F:3 * F],
                                     start=(c == 0), stop=False)
                nc.tensor.matmul(ps_in[:, :], onesrow[:, :], binrow[t][:, :],
                                 start=False, stop=True)
                ps_hn = pacc.tile([128, F], F32, name=f"pshn_{j}_{t}", tag="acc")
                for c in range(2):
                    nc.tensor.matmul(ps_hn[:, :], xh[:, 2 + c, :],
                                     whh[t][:, c, 2 * F:3 * F],
                                     start=(c == 0), stop=False)
                nc.tensor.matmul(ps_hn[:, :], onesrow[:, :], bhnrow[t][:, :],
                                 start=False, stop=True)
                rza = spool.tile([128, 2 * F], BF16, name=f"rza_{j}_{t}", tag="rza")
                nc.scalar.activation(rza[:, :], ps_rz[:, :], ACT.Sigmoid)
                tmp = spool.tile([128, F], BF16, name=f"tmp_{j}_{t}", tag="tmp")
                nc.vector.tensor_tensor(tmp[:, :], ps_hn[:, :], rza[:, 0:F],
                                        op=AOP.mult)
                t2 = spool.tile([128, F], F32, name=f"t2_{j}_{t}", tag="t2")
                nc.vector.tensor_tensor(t2[:, :], ps_in[:, :], tmp[:, :],
                                        op=AOP.add)
                nn = spool.tile([128, F], BF16, name=f"nn_{j}_{t}", tag="nn")
                nc.scalar.activation(nn[:, :], t2[:, :], ACT.Tanh)
                hm = spool.tile([128, F], BF16, name=f"hm_{j}_{t}", tag="hm")
                nc.gpsimd.tensor_tensor(hm[:, :], gf[:, :], nn[:, :],
                                        op=AOP.subtract)
                hz = spool.tile([128, F], BF16, name=f"hz_{j}_{t}", tag="hz")
                nc.vector.tensor_tensor(hz[:, :], hm[:, :], rza[:, F:2 * F],
                                        op=AOP.mult)
                if t < T - 1:
                    gf_new = spool.tile([128, F], BF16, name=f"gfn_{j}_{t}", tag="gf", bufs=4)
                    nc.vector.tensor_tensor(gf_new[:, :], hz[:, :], nn[:, :],
                                            op=AOP.add)
                    gf = gf_new
                else:
                    gout = spool.tile([128, F], F32, name=f"gout_{j}", tag="gout")
                    nc.vector.tensor_tensor(gout[:, :], hz[:, :], nn[:, :],
                                            op=AOP.add)
                    nc.scalar.dma_start(out_d[j * 128:(j + 1) * 128, :], gout[:, :])
    nc.finalize()
    return nc, ctx


def _prep_core(node_feats_bf, seg, g_lo, g_hi, NT, NSUB):
    """Stage one core's node data in the four device layouts."""
    NN = NSUB * 128
    nfaug = np.zeros((128, NT, NSUB, F + 1), NP_BF16)
    nft = np.zeros((128, NT, 2, NN), NP_BF16)
    mn = np.zeros((128, NT, 128, NSUB), NP_BF16)
    mnt = np.zeros((128, NT, NSUB, 128), NP_BF16)
    eye = np.eye(128, dtype=NP_BF16)
    for j in range(NT):
        gt = g_lo + j * 128
        if gt >= g_hi:
            continue
        ge = min(gt + 128, g_hi)
        a = int(np.searchsorted(seg, gt, 'left'))
        b = int(np.searchsorted(seg, ge, 'left'))
        cnt = b - a
        assert cnt <= NN
        tmp = np.zeros((NN, F + 1), NP_BF16)
        tmp[:cnt, 0] = 1.0
        tmp[:cnt, 1:] = node_feats_bf[a:b]
        # node n -> subtile s=n//128, partition p=n%128
        nfaug[:, j] = tmp.reshape(NSUB, 128, F + 1).transpose(1, 0, 2)
        # nft[fp, j, c, n] = nf[a+n, c*128+fp]
        nft[:, j] = np.ascontiguousarray(
            tmp[:, 1:].T.reshape(2, 128, NN).transpose(1, 0, 2))
        grel = np.full(NN, -1, np.int64)
        grel[:cnt] = seg[a:b] - gt
        oh = eye[np.clip(grel, 0, 127)] * (grel >= 0)[:, None].astype(NP_BF16)
        oh = oh.reshape(NSUB, 128, 128)          # [s, p, g]
        mn[:, j] = oh.transpose(1, 2, 0)         # [p, g, s]
        mnt[:, j] = oh.transpose(2, 0, 1)        # [g, s, p]
    return nfaug, nft, mn, mnt


def kernel(node_feats, seg_ids, Wl, bl, Wp, bp, Wih, Whh, bih, bhh):
    node_feats = np.asarray(node_feats, np.float32)
    seg = np.asarray(seg_ids).astype(np.int64)
    Wl = np.asarray(Wl, np.float32)
    bl = np.asarray(bl, np.float32)
    Wp = np.asarray(Wp, np.float32)
    bp = np.asarray(bp, np.float32)
    Wih = np.asarray(Wih, np.float32)
    Whh = np.asarray(Whh, np.float32)
    bih = np.asarray(bih, np.float32)
    bhh = np.asarray(bhh, np.float32)
    V = node_feats.shape[0]
    G = 25000

    bounds_g = [0]
    for c in range(1, NCORES):
        bounds_g.append(int(seg[c * V // NCORES]))
    bounds_g.append(G)

    NT = max((bounds_g[c + 1] - bounds_g[c] + 127) // 128 for c in range(NCORES))
    maxnodes = 1
    for c in range(NCORES):
        for gt in range(bounds_g[c], bounds_g[c + 1], 128):
            ge = min(gt + 128, bounds_g[c + 1])
            a = np.searchsorted(seg, gt, 'left')
            b = np.searchsorted(seg, ge, 'left')
            maxnodes = max(maxnodes, int(b - a))
    NSUB = (maxnodes + 127) // 128

    nc, ctx = _build_program(NT, NSUB, [float(bl[t, 0]) for t in range(T)])

    # ---- shared (replicated) weights
    shared = {
        "identb": np.eye(128, dtype=NP_BF16),
        "onesrow": np.ones((1, 128), NP_BF16),
    }
    # wl2[fp, c, t] = Wl[t, 0, 256 + c*128 + fp]
    wl2 = np.zeros((128, 2, T), np.float32)
    for t in range(T):
        for c in range(2):
            wl2[:, c, t] = Wl[t, 0, F + c * 128:F + (c + 1) * 128]
    shared["wl2"] = wl2.astype(NP_BF16)
    for t in range(T):
        shared[f"wlg{t}"] = np.broadcast_to(Wl[t, 0, :F], (128, F)).astype(NP_BF16)
        # wpt[fp, c, fo] = Wp[t, fo, c*128+fp]
        shared[f"wpt{t}"] = np.ascontiguousarray(
            Wp[t].T.reshape(2, 128, F).transpose(1, 0, 2)).astype(NP_BF16)
        shared[f"wih{t}"] = np.ascontiguousarray(
            Wih[t].T.reshape(2, 128, 3 * F).transpose(1, 0, 2)).astype(NP_BF16)
        shared[f"whh{t}"] = np.ascontiguousarray(
            Whh[t].T.reshape(2, 128, 3 * F).transpose(1, 0, 2)).astype(NP_BF16)
        shared[f"bprow{t}"] = bp[t][None, :].astype(NP_BF16)
        # elu's -1 shifted into the GRU input bias: x_gru = ctx+1 staged,
        # so bias_x -= rowsum(Wih)
        rs = Wih[t].sum(axis=1)
        shared[f"brzrow{t}"] = (bih[t, :2 * F] + bhh[t, :2 * F] - rs[:2 * F])[None, :].astype(NP_BF16)
        shared[f"binrow{t}"] = (bih[t, 2 * F:] - rs[2 * F:])[None, :].astype(NP_BF16)
        shared[f"bhnrow{t}"] = bhh[t, 2 * F:][None, :].astype(NP_BF16)

    node_feats_bf = node_feats.astype(NP_BF16)
    in_maps = []
    for c in range(NCORES):
        nfaug, nft, mn, mnt = _prep_core(
            node_feats_bf, seg, bounds_g[c], bounds_g[c + 1], NT, NSUB)
        m = dict(shared)
        m["nfaug"] = nfaug
        m["nft"] = nft
        m["mn"] = mn
        m["mnt"] = mnt
        in_maps.append(m)

    res = run_bass_kernel_spmd(nc, in_maps, core_ids=list(range(NCORES)))
    ctx.close()
    global LAST_RESULT
    LAST_RESULT = res

    out = np.zeros((G, F), np.float32)
    for c in range(NCORES):
        gc = bounds_g[c + 1] - bounds_g[c]
        out[bounds_g[c]:bounds_g[c + 1]] = res.results[c]["out"][:gc]
    return out


# revision 20
# speedup vs baseline: 2.0643x; 1.8239x over previous
"""AttentiveFP readout kernel for 8 Trainium2 NeuronCores (v4).

Graph-contiguous sharding of V=500k nodes across 8 cores (seg_ids
sorted; split at graph boundaries). All segment ops core-local, no
collectives.

Engine plan (v1 was DVE-bound at 70%; v2/v3 fixed op selection):
- node features staged by HOST as bf16 in four device layouts:
  nfaug  [128p, NT, NSUB, 257]  (col0 = valid flag, cols 1.. = nf)
  nft    [128f, NT, 2, NSUB*128] (transposed, for w01 on PE)
  mn     [128p, NT, 128, NSUB]  one-hot node->graph, g-major
  mnt    [128g, NT, NSUB, 128]  its transpose (u-gather on PE)
- per-node logits w01 = nf . wln_t : PE matmuls, nfT stationary, N=2.
- u broadcast/gather to nodes: PE matmuls MnT_s stationary @ ucol, N=1.
- e = exp(lrelu(z)) via sigmoid identity (exp table never loaded ->
  single resident ACT table set, no ACT_TABLE_LOAD thrash):
    sm = sig(-z/4); q = (sm-1)/sm = -e^{z/4}; e = (q^2)^2
- attention weights folded into the ONE-HOT (not nf): Mne = Mn * e with
  one packed tensor_tensor per timestep (innermost NSUB axis keeps the
  2x DVE mode; per-partition AP-scalar tensor_scalar is a hw slow path).
- elu(x) = relu(x) + min(e^x, 1) - 1, e^x via sigmoid ratio (x<=0 so
  no cancellation); the -1 folded into the GRU input bias on host.
- GRU/Wp biases folded into K=1 ones-row matmuls.
- v4: tiles processed in PAIRS with stage-interleaved emission so one
  tile's PE matmuls fill the other tile's cross-engine dependency gaps
  (engines execute their queues in issue order).
"""

import numpy as np
from contextlib import ExitStack

import concourse.bass as bass
import concourse.bacc as bacc
import concourse.mybir as mybir
from concourse import tile
from concourse.bass_utils import run_bass_kernel_spmd

F32 = mybir.dt.float32
BF16 = mybir.dt.bfloat16
NP_BF16 = mybir.dt.np(mybir.dt.bfloat16)
AOP = mybir.AluOpType
ACT = mybir.ActivationFunctionType
AX = mybir.AxisListType

NCORES = 8
F = 256
T = 2
LAST_RESULT = None


def _build_program(NT, NSUB, bl_vals):
    ctx = ExitStack()
    nc = bacc.Bacc("TRN2")
    nc.all_engine_barrier()

    NN = NSUB * 128  # node slots per tile

    nfaug_d = nc.dram_tensor("nfaug", [128, NT, NSUB, F + 1], BF16, kind="ExternalInput")
    nft_d = nc.dram_tensor("nft", [128, NT, 2, NN], BF16, kind="ExternalInput")
    mn_d = nc.dram_tensor("mn", [128, NT, 128, NSUB], BF16, kind="ExternalInput")
    mnt_d = nc.dram_tensor("mnt", [128, NT, NSUB, 128], BF16, kind="ExternalInput")
    identb_d = nc.dram_tensor("identb", [128, 128], BF16, kind="ExternalInput")
    onesrow_d = nc.dram_tensor("onesrow", [1, 128], BF16, kind="ExternalInput")
    wl2_d = nc.dram_tensor("wl2", [128, 2, T], BF16, kind="ExternalInput")
    wlg_d = [nc.dram_tensor(f"wlg{t}", [128, F], BF16, kind="ExternalInput") for t in range(T)]
    wpt_d = [nc.dram_tensor(f"wpt{t}", [128, 2, F], BF16, kind="ExternalInput") for t in range(T)]
    wih_d = [nc.dram_tensor(f"wih{t}", [128, 2, 3 * F], BF16, kind="ExternalInput") for t in range(T)]
    whh_d = [nc.dram_tensor(f"whh{t}", [128, 2, 3 * F], BF16, kind="ExternalInput") for t in range(T)]
    bprow_d = [nc.dram_tensor(f"bprow{t}", [1, F], BF16, kind="ExternalInput") for t in range(T)]
    brzrow_d = [nc.dram_tensor(f"brzrow{t}", [1, 2 * F], BF16, kind="ExternalInput") for t in range(T)]
    binrow_d = [nc.dram_tensor(f"binrow{t}", [1, F], BF16, kind="ExternalInput") for t in range(T)]
    bhnrow_d = [nc.dram_tensor(f"bhnrow{t}", [1, F], BF16, kind="ExternalInput") for t in range(T)]
    out_d = nc.dram_tensor("out", [NT * 128, F], F32, kind="ExternalOutput")

    with tile.TileContext(nc) as tc:
      with tc.sbuf_pool(name="const", bufs=1) as cpool, \
           tc.sbuf_pool(name="work", bufs=4) as wpool, \
           tc.sbuf_pool(name="scr", bufs=3) as scrpool, \
           tc.sbuf_pool(name="small", bufs=3) as spool, \
           tc.psum_pool(name="pacc", bufs=4) as pacc, \
           tc.psum_pool(name="prz", bufs=2) as prz, \
           tc.psum_pool(name="ptiny", bufs=2) as ptiny:

        identb = cpool.tile_from(identb_d[:, :], name="identb")
        onesrow = cpool.tile_from(onesrow_d[:, :], name="onesrow")
        wl2 = cpool.tile_from(wl2_d[:, :, :], name="wl2")
        wlg = [cpool.tile_from(wlg_d[t][:, :], name=f"wlg{t}") for t in range(T)]
        wpt = [cpool.tile_from(wpt_d[t][:, :, :], name=f"wpt{t}") for t in range(T)]
        wih = [cpool.tile_from(wih_d[t][:, :, :], name=f"wih{t}") for t in range(T)]
        whh = [cpool.tile_from(whh_d[t][:, :, :], name=f"whh{t}") for t in range(T)]
        bprow = [cpool.tile_from(bprow_d[t][:, :], name=f"bprow{t}") for t in range(T)]
        brzrow = [cpool.tile_from(brzrow_d[t][:, :], name=f"brzrow{t}") for t in range(T)]
        binrow = [cpool.tile_from(binrow_d[t][:, :], name=f"binrow{t}") for t in range(T)]
        bhnrow = [cpool.tile_from(bhnrow_d[t][:, :], name=f"bhnrow{t}") for t in range(T)]

        S = {}  # per-tile live state

        def emit_dma(j):
            s = S[j] = {}
            s["nfaug"] = wpool.tile([128, NSUB, F + 1], BF16, name=f"nfaug{j}", tag="nfaug")
            s["nft"] = wpool.tile([128, 2, NN], BF16, name=f"nft{j}", tag="nft", bufs=2)
            s["mn"] = wpool.tile([128, 128, NSUB], BF16, name=f"mn{j}", tag="mn")
            s["mnt"] = wpool.tile([128, NSUB, 128], BF16, name=f"mnt{j}", tag="mnt")
            nc.sync.dma_start(s["nfaug"][:, :, :], nfaug_d[:, j, :, :])
            nc.scalar.dma_start(s["nft"][:, :, :], nft_d[:, j, :, :])
            nc.gpsimd.dma_start(s["mn"][:, :, :], mn_d[:, j, :, :])
            nc.sync.dma_start(s["mnt"][:, :, :], mnt_d[:, j, :, :])

        def emit_g0(j):
            s = S[j]
            ps_g0 = pacc.tile([128, F], F32, name=f"psg0_{j}", tag="acc")
            for k in range(NSUB):
                nc.tensor.matmul(ps_g0[:, :], s["mn"][:, :, k],
                                 s["nfaug"][:, k, 1:F + 1],
                                 start=(k == 0), stop=(k == NSUB - 1))
            gf = spool.tile([128, F], BF16, name=f"gf0_{j}", tag="gf", bufs=8)
            nc.scalar.copy(gf[:, :], ps_g0[:, :])
            s["gf"] = gf

        def emit_w01(j):
            s = S[j]
            ps_w01 = ptiny.tile([128, NSUB, T], F32, name=f"psw01_{j}", tag="tiny")
            for k in range(NSUB):
                for c in range(2):
                    nc.tensor.matmul(ps_w01[:, k, :],
                                     s["nft"][:, c, k * 128:(k + 1) * 128],
                                     wl2[:, c, :],
                                     start=(c == 0), stop=(c == 1))
            w01sb = spool.tile([128, T, NSUB], BF16, name=f"w01_{j}", tag="w01")
            nc.vector.tensor_copy(w01sb[:, :, :],
                                  ps_w01[:, :, :].transpose((0, 2, 1)))
            s["w01"] = w01sb

        def emit_u(j, t):
            s = S[j]
            uscr = spool.tile([128, F], BF16, name=f"uscr_{j}_{t}", tag="uscr")
            ucol = spool.tile([128, 1], F32, name=f"ucol_{j}_{t}", tag="ucol")
            nc.vector.scalar_tensor_tensor(
                uscr[:, :], s["gf"][:, :], 0.0, wlg[t][:, :],
                op0=AOP.max, op1=AOP.mult, accum_out=ucol[:, :])
            ucolb = spool.tile([128, 1], BF16, name=f"ucolb_{j}_{t}", tag="ucolb")
            nc.scalar.activation(ucolb[:, :], ucol[:, :], ACT.Copy,
                                 bias=float(bl_vals[t]))
            s["ucolb"] = ucolb

        def emit_ubcv(j, t):
            s = S[j]
            ps_ubcv = ptiny.tile([128, NSUB], F32, name=f"psub_{j}_{t}", tag="tiny")
            for k in range(NSUB):
                nc.tensor.matmul(ps_ubcv[:, k:k + 1], s["mnt"][:, k, :],
                                 s["ucolb"][:, :], start=True, stop=True)
            s["ps_ubcv"] = ps_ubcv

        def emit_echain(j, t):
            s = S[j]
            zt = spool.tile([128, NSUB], F32, name=f"zt_{j}_{t}", tag="zt")
            nc.vector.tensor_tensor(zt[:, :], s["ps_ubcv"][:, :],
                                    s["w01"][:, t, :], op=AOP.add)
            zl = spool.tile([128, NSUB], F32, name=f"zl_{j}_{t}", tag="zl")
            nc.scalar.activation(zl[:, :], zt[:, :], ACT.Prelu, alpha=0.01)
            sm = spool.tile([128, NSUB], F32, name=f"sm_{j}_{t}", tag="sm")
            nc.scalar.activation(sm[:, :], zl[:, :], ACT.Sigmoid, scale=-0.25)
            rsm = spool.tile([128, NSUB], F32, name=f"rsm_{j}_{t}", tag="rsm")
            nc.vector.reciprocal(rsm[:, :], sm[:, :])
            q4 = spool.tile([128, NSUB], F32, name=f"q4_{j}_{t}", tag="q4")
            nc.vector.tensor_scalar(q4[:, :], rsm[:, :], -1.0, 1.0,
                                    op0=AOP.mult, op1=AOP.add)
            q2 = spool.tile([128, NSUB], F32, name=f"q2_{j}_{t}", tag="q2")
            nc.vector.tensor_tensor(q2[:, :], q4[:, :], q4[:, :], op=AOP.mult)
            ebf = spool.tile([128, NSUB], BF16, name=f"ebf_{j}_{t}", tag="ebf")
            nc.vector.tensor_tensor(ebf[:, :], q2[:, :], q2[:, :], op=AOP.mult)
            mne = scrpool.tile([128, 128, NSUB], BF16, name=f"mne_{j}_{t}", tag="mne")
            nc.vector.tensor_tensor(
                mne[:, :, :], s["mn"][:, :, :],
                ebf[:, :].unsqueeze(1).broadcast_to((128, 128, NSUB)),
                op=AOP.mult)
            s["mne"] = mne

        def emit_ds(j, t):
            s = S[j]
            ps_ds = pacc.tile([128, F + 1], F32, name=f"psds_{j}_{t}", tag="acc")
            for k in range(NSUB):
                nc.tensor.matmul(ps_ds[:, :], s["mne"][:, :, k],
                                 s["nfaug"][:, k, :],
                                 start=(k == 0), stop=(k == NSUB - 1))
            s["ps_ds"] = ps_ds

        def emit_stl(j, t):
            s = S[j]
            dplus = spool.tile([128, 1], F32, name=f"dp_{j}_{t}", tag="dp")
            nc.vector.tensor_scalar(dplus[:, :], s["ps_ds"][:, 0:1], 1e-30, None,
                                    op0=AOP.max)
            recd = spool.tile([128, 1], F32, name=f"recd_{j}_{t}", tag="recd")
            nc.vector.reciprocal(recd[:, :], dplus[:, :])
            stl = spool.tile([128, F], BF16, name=f"stl_{j}_{t}", tag="stl")
            nc.scalar.activation(stl[:, :], s["ps_ds"][:, 1:F + 1], ACT.Copy,
                                 scale=recd[:, :])
            s["stl"] = stl

        def emit_ctx(j, t):
            s = S[j]
            ps_st = ptiny.tile([128, 2, 128], BF16, name=f"psst_{j}_{t}", tag="tiny")
            for c in range(2):
                nc.tensor.transpose(ps_st[:, c, :],
                                    s["stl"][:, c * 128:(c + 1) * 128],
                                    identb[:, :])
            stT = spool.tile([128, 2, 128], BF16, name=f"stT_{j}_{t}", tag="stT")
            nc.vector.tensor_copy(stT[:, :, :], ps_st[:, :, :])
            ps_ctx = pacc.tile([128, F], F32, name=f"psctx_{j}_{t}", tag="acc")
            nc.tensor.matmul(ps_ctx[:, :], stT[:, 0, :], wpt[t][:, 0, :],
                             start=True, stop=False)
            nc.tensor.matmul(ps_ctx[:, :], stT[:, 1, :], wpt[t][:, 1, :],
                             start=False, stop=False)
            nc.tensor.matmul(ps_ctx[:, :], onesrow[:, :], bprow[t][:, :],
                             start=False, stop=True)
            sg = spool.tile([128, F], BF16, name=f"sg_{j}_{t}", tag="sg")
            nc.scalar.activation(sg[:, :], ps_ctx[:, :], ACT.Sigmoid)
            omy = spool.tile([128, F], F32, name=f"omy_{j}_{t}", tag="omy")
            nc.gpsimd.tensor_scalar(omy[:, :], sg[:, :], -1.0, 1.0,
                                    op0=AOP.mult, op1=AOP.add)
            romy = spool.tile([128, F], F32, name=f"romy_{j}_{t}", tag="romy")
            nc.vector.reciprocal(romy[:, :], omy[:, :])
            exn = spool.tile([128, F], BF16, name=f"exn_{j}_{t}", tag="exn")
            nc.vector.tensor_tensor(exn[:, :], sg[:, :], romy[:, :], op=AOP.mult)
            exm = spool.tile([128, F], BF16, name=f"exm_{j}_{t}", tag="exm")
            nc.gpsimd.tensor_scalar(exm[:, :], exn[:, :], 1.0, None, op0=AOP.min)
            ctxb = spool.tile([128, F], BF16, name=f"ctxb_{j}_{t}", tag="ctxb")
            nc.vector.scalar_tensor_tensor(
                ctxb[:, :], ps_ctx[:, :], 0.0, exm[:, :],
                op0=AOP.max, op1=AOP.add)
            s["ctxb"] = ctxb

        def emit_gru(j, t):
            s = S[j]
            gf = s["gf"]
            ps_tr = ptiny.tile([128, 4, 128], BF16, name=f"pstr_{j}_{t}", tag="tiny")
            for c in range(2):
                nc.tensor.transpose(ps_tr[:, c, :],
                                    s["ctxb"][:, c * 128:(c + 1) * 128],
                                    identb[:, :])
                nc.tensor.transpose(ps_tr[:, 2 + c, :],
                                    gf[:, c * 128:(c + 1) * 128],
                                    identb[:, :])
            xh = spool.tile([128, 4, 128], BF16, name=f"xh_{j}_{t}", tag="xh")
            nc.vector.tensor_copy(xh[:, :, :], ps_tr[:, :, :])
            ps_rz = prz.tile([128, 2 * F], F32, name=f"psrz_{j}_{t}", tag="rz")
            mmi = 0
            for base, wt in ((0, wih[t]), (2, whh[t])):
                for c in range(2):
                    nc.tensor.matmul(ps_rz[:, :], xh[:, base + c, :],
                                     wt[:, c, 0:2 * F],
                                     start=(mmi == 0), stop=False)
                    mmi += 1
            nc.tensor.matmul(ps_rz[:, :], onesrow[:, :], brzrow[t][:, :],
                             start=False, stop=True)
            ps_in = pacc.tile([128, F], F32, name=f"psin_{j}_{t}", tag="acc")
            for c in range(2):
                nc.tensor.matmul(ps_in[:, :], xh[:, c, :],
                                 wih[t][:, c, 2 * F:3 * F],
                                 start=(c == 0), stop=False)
            nc.tensor.matmul(ps_in[:, :], onesrow[:, :], binrow[t][:, :],
                             start=False, stop=True)
            ps_hn = pacc.tile([128, F], F32, name=f"pshn_{j}_{t}", tag="acc")
            for c in range(2):
                nc.tensor.matmul(ps_hn[:, :], xh[:, 2 + c, :],
                                 whh[t][:, c, 2 * F:3 * F],
                                 start=(c == 0), stop=False)
            nc.tensor.matmul(ps_hn[:, :], onesrow[:, :], bhnrow[t][:, :],
                             start=False, stop=True)
            rza = spool.tile([128, 2 * F], BF16, name=f"rza_{j}_{t}", tag="rza")
            nc.scalar.activation(rza[:, :], ps_rz[:, :], ACT.Sigmoid)
            tmp = spool.tile([128, F], BF16, name=f"tmp_{j}_{t}", tag="tmp")
            nc.vector.tensor_tensor(tmp[:, :], ps_hn[:, :], rza[:, 0:F],
                                    op=AOP.mult)
            t2 = spool.tile([128, F], F32, name=f"t2_{j}_{t}", tag="t2")
            nc.vector.tensor_tensor(t2[:, :], ps_in[:, :], tmp[:, :], op=AOP.add)
            nn = spool.tile([128, F], BF16, name=f"nn_{j}_{t}", tag="nn")
            nc.scalar.activation(nn[:, :], t2[:, :], ACT.Tanh)
            hm = spool.tile([128, F], BF16, name=f"hm_{j}_{t}", tag="hm")
            nc.gpsimd.tensor_tensor(hm[:, :], gf[:, :], nn[:, :], op=AOP.subtract)
            hz = spool.tile([128, F], BF16, name=f"hz_{j}_{t}", tag="hz")
            nc.vector.tensor_tensor(hz[:, :], hm[:, :], rza[:, F:2 * F],
                                    op=AOP.mult)
            if t < T - 1:
                gf_new = spool.tile([128, F], BF16, name=f"gfn_{j}_{t}", tag="gf", bufs=8)
                nc.vector.tensor_tensor(gf_new[:, :], hz[:, :], nn[:, :],
                                        op=AOP.add)
                s["gf"] = gf_new
            else:
                gout = spool.tile([128, F], F32, name=f"gout_{j}", tag="gout")
                nc.vector.tensor_tensor(gout[:, :], hz[:, :], nn[:, :],
                                        op=AOP.add)
                nc.scalar.dma_start(out_d[j * 128:(j + 1) * 128, :], gout[:, :])
                del S[j]

        pairs = [tuple(j for j in (j0, j0 + 1) if j < NT)
                 for j0 in range(0, NT, 2)]
        for j in pairs[0]:
            emit_dma(j)
        for p, pair in enumerate(pairs):
            if p + 1 < len(pairs):
                for j in pairs[p + 1]:
                    emit_dma(j)
            for j in pair:
                emit_g0(j)
            for j in pair:
                emit_w01(j)
            for t in range(T):
                for j in pair:
                    emit_u(j, t)
                for j in pair:
                    emit_ubcv(j, t)
                for j in pair:
                    emit_echain(j, t)
                for j in pair:
                    emit_ds(j, t)
                for j in pair:
                    emit_stl(j, t)
                for j in pair:
                    emit_ctx(j, t)
                for j in pair:
                    emit_gru(j, t)
    nc.finalize()
    return nc, ctx


def _prep_core(node_feats_bf, seg, g_lo, g_hi, NT, NSUB):
    """Stage one core's node data in the four device layouts."""
    NN = NSUB * 128
    nfaug = np.zeros((128, NT, NSUB, F + 1), NP_BF16)
    nft = np.zeros((128, NT, 2, NN), NP_BF16)
    mn = np.zeros((128, NT, 128, NSUB), NP_BF16)
    mnt = np.zeros((128, NT, NSUB, 128), NP_BF16)
    eye = np.eye(128, dtype=NP_BF16)
    for j in range(NT):
        gt = g_lo + j * 128
        if gt >= g_hi:
            continue
        ge = min(gt + 128, g_hi)
        a = int(np.searchsorted(seg, gt, 'left'))
        b = int(np.searchsorted(seg, ge, 'left'))
        cnt = b - a
        assert cnt <= NN
        tmp = np.zeros((NN, F + 1), NP_BF16)
        tmp[:cnt, 0] = 1.0
        tmp[:cnt, 1:] = node_feats_bf[a:b]
        # node n -> subtile s=n//128, partition p=n%128
        nfaug[:, j] = tmp.reshape(NSUB, 128, F + 1).transpose(1, 0, 2)
        # nft[fp, j, c, n] = nf[a+n, c*128+fp]
        nft[:, j] = np.ascontiguousarray(
            tmp[:, 1:].T.reshape(2, 128, NN).transpose(1, 0, 2))
        grel = np.full(NN, -1, np.int64)
        grel[:cnt] = seg[a:b] - gt
        oh = eye[np.clip(grel, 0, 127)] * (grel >= 0)[:, None].astype(NP_BF16)
        oh = oh.reshape(NSUB, 128, 128)          # [s, p, g]
        mn[:, j] = oh.transpose(1, 2, 0)         # [p, g, s]
        mnt[:, j] = oh.transpose(2, 0, 1)        # [g, s, p]
    return nfaug, nft, mn, mnt


def kernel(node_feats, seg_ids, Wl, bl, Wp, bp, Wih, Whh, bih, bhh):
    node_feats = np.asarray(node_feats, np.float32)
    seg = np.asarray(seg_ids).astype(np.int64)
    Wl = np.asarray(Wl, np.float32)
    bl = np.asarray(bl, np.float32)
    Wp = np.asarray(Wp, np.float32)
    bp = np.asarray(bp, np.float32)
    Wih = np.asarray(Wih, np.float32)
    Whh = np.asarray(Whh, np.float32)
    bih = np.asarray(bih, np.float32)
    bhh = np.asarray(bhh, np.float32)
    V = node_feats.shape[0]
    G = 25000

    bounds_g = [0]
    for c in range(1, NCORES):
        bounds_g.append(int(seg[c * V // NCORES]))
    bounds_g.append(G)

    NT = max((bounds_g[c + 1] - bounds_g[c] + 127) // 128 for c in range(NCORES))
    maxnodes = 1
    for c in range(NCORES):
        for gt in range(bounds_g[c], bounds_g[c + 1], 128):
            ge = min(gt + 128, bounds_g[c + 1])
            a = np.searchsorted(seg, gt, 'left')
            b = np.searchsorted(seg, ge, 'left')
            maxnodes = max(maxnodes, int(b - a))
    NSUB = (maxnodes + 127) // 128

    nc, ctx = _build_program(NT, NSUB, [float(bl[t, 0]) for t in range(T)])

    shared = {
        "identb": np.eye(128, dtype=NP_BF16),
        "onesrow": np.ones((1, 128), NP_BF16),
    }
    wl2 = np.zeros((128, 2, T), np.float32)
    for t in range(T):
        for c in range(2):
            wl2[:, c, t] = Wl[t, 0, F + c * 128:F + (c + 1) * 128]
    shared["wl2"] = wl2.astype(NP_BF16)
    for t in range(T):
        shared[f"wlg{t}"] = np.broadcast_to(Wl[t, 0, :F], (128, F)).astype(NP_BF16)
        shared[f"wpt{t}"] = np.ascontiguousarray(
            Wp[t].T.reshape(2, 128, F).transpose(1, 0, 2)).astype(NP_BF16)
        shared[f"wih{t}"] = np.ascontiguousarray(
            Wih[t].T.reshape(2, 128, 3 * F).transpose(1, 0, 2)).astype(NP_BF16)
        shared[f"whh{t}"] = np.ascontiguousarray(
            Whh[t].T.reshape(2, 128, 3 * F).transpose(1, 0, 2)).astype(NP_BF16)
        shared[f"bprow{t}"] = bp[t][None, :].astype(NP_BF16)
        # elu's -1 shifted into the GRU input bias: x_gru = ctx+1 staged,
        # so bias_x -= rowsum(Wih)
        rs = Wih[t].sum(axis=1)
        shared[f"brzrow{t}"] = (bih[t, :2 * F] + bhh[t, :2 * F] - rs[:2 * F])[None, :].astype(NP_BF16)
        shared[f"binrow{t}"] = (bih[t, 2 * F:] - rs[2 * F:])[None, :].astype(NP_BF16)
        shared[f"bhnrow{t}"] = bhh[t, 2 * F:][None, :].astype(NP_BF16)

    node_feats_bf = node_feats.astype(NP_BF16)
    in_maps = []
    for c in range(NCORES):
        nfaug, nft, mn, mnt = _prep_core(
            node_feats_bf, seg, bounds_g[c], bounds_g[c + 1], NT, NSUB)
        m = dict(shared)
        m["nfaug"] = nfaug
        m["nft"] = nft
        m["mn"] = mn
        m["mnt"] = mnt
        in_maps.append(m)

    res = run_bass_kernel_spmd(nc, in_maps, core_ids=list(range(NCORES)))
    ctx.close()
    global LAST_RESULT
    LAST_RESULT = res

    out = np.zeros((G, F), np.float32)
    for c in range(NCORES):
        gc = bounds_g[c + 1] - bounds_g[c]
        out[bounds_g[c]:bounds_g[c + 1]] = res.results[c]["out"][:gc]
    return out


# revision 22
# speedup vs baseline: 2.4803x; 1.2015x over previous
"""AttentiveFP readout kernel for 8 Trainium2 NeuronCores (v4).

Graph-contiguous sharding of V=500k nodes across 8 cores (seg_ids
sorted; split at graph boundaries). All segment ops core-local, no
collectives.

Engine plan (v1 was DVE-bound at 70%; v2/v3 fixed op selection):
- node features staged by HOST as bf16 in four device layouts:
  nfaug  [128p, NT, NSUB, 257]  (col0 = valid flag, cols 1.. = nf)
  nft    [128f, NT, 2, NSUB*128] (transposed, for w01 on PE)
  mn     [128p, NT, 128, NSUB]  one-hot node->graph, g-major
  mnt    [128g, NT, NSUB, 128]  its transpose (u-gather on PE)
- per-node logits w01 = nf . wln_t : PE matmuls, nfT stationary, N=2.
- u broadcast/gather to nodes: PE matmuls MnT_s stationary @ ucol, N=1.
- e = exp(lrelu(z)) via sigmoid identity (exp table never loaded ->
  single resident ACT table set, no ACT_TABLE_LOAD thrash):
    sm = sig(-z/4); q = (sm-1)/sm = -e^{z/4}; e = (q^2)^2
- attention weights folded into the ONE-HOT (not nf): Mne = Mn * e with
  one packed tensor_tensor per timestep (innermost NSUB axis keeps the
  2x DVE mode; per-partition AP-scalar tensor_scalar is a hw slow path).
- elu(x) = relu(x) + min(e^x, 1) - 1, e^x via sigmoid ratio (x<=0 so
  no cancellation); the -1 folded into the GRU input bias on host.
- GRU/Wp biases folded into K=1 ones-row matmuls.
- v4: tiles processed in PAIRS with stage-interleaved emission so one
  tile's PE matmuls fill the other tile's cross-engine dependency gaps
  (engines execute their queues in issue order).
"""

import numpy as np
from contextlib import ExitStack

import concourse.bass as bass
import concourse.bacc as bacc
import concourse.mybir as mybir
from concourse import tile
from concourse.bass_utils import run_bass_kernel_spmd

F32 = mybir.dt.float32
BF16 = mybir.dt.bfloat16
NP_BF16 = mybir.dt.np(mybir.dt.bfloat16)
AOP = mybir.AluOpType
ACT = mybir.ActivationFunctionType
AX = mybir.AxisListType

NCORES = 8
F = 256
T = 2
LAST_RESULT = None


def _build_program(NT, NSUB, bl_vals):
    ctx = ExitStack()
    nc = bacc.Bacc("TRN2")
    nc.all_engine_barrier()

    NN = NSUB * 128  # node slots per tile

    nfaug_d = nc.dram_tensor("nfaug", [128, NT, NSUB, F + 1], BF16, kind="ExternalInput")
    nft_d = nc.dram_tensor("nft", [128, NT, 2, NN], BF16, kind="ExternalInput")
    mn_d = nc.dram_tensor("mn", [128, NT, 128, NSUB], BF16, kind="ExternalInput")
    mnt_d = nc.dram_tensor("mnt", [128, NT, NSUB, 128], BF16, kind="ExternalInput")
    identb_d = nc.dram_tensor("identb", [128, 128], BF16, kind="ExternalInput")
    onesrow_d = nc.dram_tensor("onesrow", [1, 128], BF16, kind="ExternalInput")
    wl2_d = nc.dram_tensor("wl2", [128, 2, T], BF16, kind="ExternalInput")
    wlg_d = [nc.dram_tensor(f"wlg{t}", [128, F], BF16, kind="ExternalInput") for t in range(T)]
    wpt_d = [nc.dram_tensor(f"wpt{t}", [128, 2, F], BF16, kind="ExternalInput") for t in range(T)]
    wih_d = [nc.dram_tensor(f"wih{t}", [128, 2, 3 * F], BF16, kind="ExternalInput") for t in range(T)]
    whh_d = [nc.dram_tensor(f"whh{t}", [128, 2, 3 * F], BF16, kind="ExternalInput") for t in range(T)]
    bprow_d = [nc.dram_tensor(f"bprow{t}", [1, F], BF16, kind="ExternalInput") for t in range(T)]
    brzrow_d = [nc.dram_tensor(f"brzrow{t}", [1, 2 * F], BF16, kind="ExternalInput") for t in range(T)]
    binrow_d = [nc.dram_tensor(f"binrow{t}", [1, F], BF16, kind="ExternalInput") for t in range(T)]
    bhnrow_d = [nc.dram_tensor(f"bhnrow{t}", [1, F], BF16, kind="ExternalInput") for t in range(T)]
    out_d = nc.dram_tensor("out", [NT * 128, F], F32, kind="ExternalOutput")

    with tile.TileContext(nc) as tc:
      with tc.sbuf_pool(name="const", bufs=1) as cpool, \
           tc.sbuf_pool(name="work", bufs=4) as wpool, \
           tc.sbuf_pool(name="scr", bufs=3) as scrpool, \
           tc.sbuf_pool(name="small", bufs=3) as spool, \
           tc.psum_pool(name="pacc", bufs=4) as pacc, \
           tc.psum_pool(name="prz", bufs=2) as prz, \
           tc.psum_pool(name="ptiny", bufs=2) as ptiny:

        identb = cpool.tile_from(identb_d[:, :], name="identb")
        onesrow = cpool.tile_from(onesrow_d[:, :], name="onesrow")
        wl2 = cpool.tile_from(wl2_d[:, :, :], name="wl2")
        wlg = [cpool.tile_from(wlg_d[t][:, :], name=f"wlg{t}") for t in range(T)]
        wpt = [cpool.tile_from(wpt_d[t][:, :, :], name=f"wpt{t}") for t in range(T)]
        wih = [cpool.tile_from(wih_d[t][:, :, :], name=f"wih{t}") for t in range(T)]
        whh = [cpool.tile_from(whh_d[t][:, :, :], name=f"whh{t}") for t in range(T)]
        bprow = [cpool.tile_from(bprow_d[t][:, :], name=f"bprow{t}") for t in range(T)]
        brzrow = [cpool.tile_from(brzrow_d[t][:, :], name=f"brzrow{t}") for t in range(T)]
        binrow = [cpool.tile_from(binrow_d[t][:, :], name=f"binrow{t}") for t in range(T)]
        bhnrow = [cpool.tile_from(bhnrow_d[t][:, :], name=f"bhnrow{t}") for t in range(T)]

        S = {}  # per-tile live state

        def emit_dma(j):
            s = S[j] = {}
            s["nfaug"] = wpool.tile([128, NSUB, F + 1], BF16, name=f"nfaug{j}", tag="nfaug")
            s["nft"] = wpool.tile([128, 2, NN], BF16, name=f"nft{j}", tag="nft", bufs=2)
            s["mn"] = wpool.tile([128, 128, NSUB], BF16, name=f"mn{j}", tag="mn")
            s["mnt"] = wpool.tile([128, NSUB, 128], BF16, name=f"mnt{j}", tag="mnt")
            nc.sync.dma_start(s["nfaug"][:, :, :], nfaug_d[:, j, :, :])
            nc.scalar.dma_start(s["nft"][:, :, :], nft_d[:, j, :, :])
            nc.gpsimd.dma_start(s["mn"][:, :, :], mn_d[:, j, :, :])
            nc.sync.dma_start(s["mnt"][:, :, :], mnt_d[:, j, :, :])

        def emit_g0(j):
            s = S[j]
            ps_g0 = pacc.tile([128, F], F32, name=f"psg0_{j}", tag="acc")
            for k in range(NSUB):
                nc.tensor.matmul(ps_g0[:, :], s["mn"][:, :, k],
                                 s["nfaug"][:, k, 1:F + 1],
                                 start=(k == 0), stop=(k == NSUB - 1))
            gf = spool.tile([128, F], BF16, name=f"gf0_{j}", tag="gf", bufs=8)
            nc.scalar.copy(gf[:, :], ps_g0[:, :])
            s["gf"] = gf

        def emit_w01(j):
            s = S[j]
            ps_w01 = ptiny.tile([128, NSUB, T], F32, name=f"psw01_{j}", tag="tiny")
            for k in range(NSUB):
                for c in range(2):
                    nc.tensor.matmul(ps_w01[:, k, :],
                                     s["nft"][:, c, k * 128:(k + 1) * 128],
                                     wl2[:, c, :],
                                     start=(c == 0), stop=(c == 1))
            w01sb = spool.tile([128, T, NSUB], BF16, name=f"w01_{j}", tag="w01")
            nc.vector.tensor_copy(w01sb[:, :, :],
                                  ps_w01[:, :, :].transpose((0, 2, 1)))
            s["w01"] = w01sb

        def emit_u(j, t):
            s = S[j]
            uscr = spool.tile([128, F], BF16, name=f"uscr_{j}_{t}", tag="uscr")
            ucol = spool.tile([128, 1], F32, name=f"ucol_{j}_{t}", tag="ucol")
            nc.vector.scalar_tensor_tensor(
                uscr[:, :], s["gf"][:, :], 0.0, wlg[t][:, :],
                op0=AOP.max, op1=AOP.mult, accum_out=ucol[:, :])
            ucolb = spool.tile([128, 1], BF16, name=f"ucolb_{j}_{t}", tag="ucolb")
            nc.scalar.activation(ucolb[:, :], ucol[:, :], ACT.Copy,
                                 bias=float(bl_vals[t]))
            s["ucolb"] = ucolb

        def emit_ubcv(j, t):
            s = S[j]
            ps_ubcv = ptiny.tile([128, NSUB], F32, name=f"psub_{j}_{t}", tag="tiny")
            for k in range(NSUB):
                nc.tensor.matmul(ps_ubcv[:, k:k + 1], s["mnt"][:, k, :],
                                 s["ucolb"][:, :], start=True, stop=True)
            s["ps_ubcv"] = ps_ubcv

        def emit_echain(j, t):
            s = S[j]
            zt = spool.tile([128, NSUB], F32, name=f"zt_{j}_{t}", tag="zt")
            nc.vector.tensor_tensor(zt[:, :], s["ps_ubcv"][:, :],
                                    s["w01"][:, t, :], op=AOP.add)
            zl = spool.tile([128, NSUB], F32, name=f"zl_{j}_{t}", tag="zl")
            nc.scalar.activation(zl[:, :], zt[:, :], ACT.Prelu, alpha=0.01)
            sm = spool.tile([128, NSUB], F32, name=f"sm_{j}_{t}", tag="sm")
            nc.scalar.activation(sm[:, :], zl[:, :], ACT.Sigmoid, scale=-0.25)
            rsm = spool.tile([128, NSUB], F32, name=f"rsm_{j}_{t}", tag="rsm")
            nc.vector.reciprocal(rsm[:, :], sm[:, :])
            q4 = spool.tile([128, NSUB], F32, name=f"q4_{j}_{t}", tag="q4")
            nc.vector.tensor_scalar(q4[:, :], rsm[:, :], -1.0, 1.0,
                                    op0=AOP.mult, op1=AOP.add)
            q2 = spool.tile([128, NSUB], F32, name=f"q2_{j}_{t}", tag="q2")
            nc.vector.tensor_tensor(q2[:, :], q4[:, :], q4[:, :], op=AOP.mult)
            ebf = spool.tile([128, NSUB], BF16, name=f"ebf_{j}_{t}", tag="ebf")
            nc.vector.tensor_tensor(ebf[:, :], q2[:, :], q2[:, :], op=AOP.mult)
            mne = scrpool.tile([128, 128, NSUB], BF16, name=f"mne_{j}_{t}", tag="mne")
            nc.vector.tensor_tensor(
                mne[:, :, :], s["mn"][:, :, :],
                ebf[:, :].unsqueeze(1).broadcast_to((128, 128, NSUB)),
                op=AOP.mult)
            s["mne"] = mne

        def emit_ds(j, t):
            s = S[j]
            ps_ds = pacc.tile([128, F + 1], F32, name=f"psds_{j}_{t}", tag="acc")
            for k in range(NSUB):
                nc.tensor.matmul(ps_ds[:, :], s["mne"][:, :, k],
                                 s["nfaug"][:, k, :],
                                 start=(k == 0), stop=(k == NSUB - 1))
            s["ps_ds"] = ps_ds

        def emit_stl(j, t):
            s = S[j]
            dplus = spool.tile([128, 1], F32, name=f"dp_{j}_{t}", tag="dp")
            nc.vector.tensor_scalar(dplus[:, :], s["ps_ds"][:, 0:1], 1e-30, None,
                                    op0=AOP.max)
            recd = spool.tile([128, 1], F32, name=f"recd_{j}_{t}", tag="recd")
            nc.vector.reciprocal(recd[:, :], dplus[:, :])
            stl = spool.tile([128, F], BF16, name=f"stl_{j}_{t}", tag="stl")
            nc.scalar.activation(stl[:, :], s["ps_ds"][:, 1:F + 1], ACT.Copy,
                                 scale=recd[:, :])
            s["stl"] = stl

        def emit_ctx(j, t):
            s = S[j]
            ps_st = ptiny.tile([128, 2, 128], BF16, name=f"psst_{j}_{t}", tag="tiny")
            for c in range(2):
                nc.tensor.transpose(ps_st[:, c, :],
                                    s["stl"][:, c * 128:(c + 1) * 128],
                                    identb[:, :])
            stT = spool.tile([128, 2, 128], BF16, name=f"stT_{j}_{t}", tag="stT")
            nc.vector.tensor_copy(stT[:, :, :], ps_st[:, :, :])
            ps_ctx = pacc.tile([128, F], F32, name=f"psctx_{j}_{t}", tag="acc")
            nc.tensor.matmul(ps_ctx[:, :], stT[:, 0, :], wpt[t][:, 0, :],
                             start=True, stop=False)
            nc.tensor.matmul(ps_ctx[:, :], stT[:, 1, :], wpt[t][:, 1, :],
                             start=False, stop=False)
            nc.tensor.matmul(ps_ctx[:, :], onesrow[:, :], bprow[t][:, :],
                             start=False, stop=True)
            sg = spool.tile([128, F], BF16, name=f"sg_{j}_{t}", tag="sg")
            nc.scalar.activation(sg[:, :], ps_ctx[:, :], ACT.Sigmoid)
            omy = spool.tile([128, F], F32, name=f"omy_{j}_{t}", tag="omy")
            nc.scalar.activation(omy[:, :], sg[:, :], ACT.Copy,
                                 scale=-1.0, bias=1.0)
            romy = spool.tile([128, F], F32, name=f"romy_{j}_{t}", tag="romy")
            nc.vector.reciprocal_approx_fast(out=romy[:, :], in_=omy[:, :])
            exn = spool.tile([128, F], BF16, name=f"exn_{j}_{t}", tag="exn")
            nc.vector.tensor_tensor(exn[:, :], sg[:, :], romy[:, :], op=AOP.mult)
            exm = spool.tile([128, F], BF16, name=f"exm_{j}_{t}", tag="exm")
            nc.vector.tensor_scalar(exm[:, :], exn[:, :], 1.0, None, op0=AOP.min)
            ctxb = spool.tile([128, F], BF16, name=f"ctxb_{j}_{t}", tag="ctxb")
            nc.vector.scalar_tensor_tensor(
                ctxb[:, :], ps_ctx[:, :], 0.0, exm[:, :],
                op0=AOP.max, op1=AOP.add)
            s["ctxb"] = ctxb

        def emit_gru(j, t):
            s = S[j]
            gf = s["gf"]
            ps_tr = ptiny.tile([128, 4, 128], BF16, name=f"pstr_{j}_{t}", tag="tiny")
            for c in range(2):
                nc.tensor.transpose(ps_tr[:, c, :],
                                    s["ctxb"][:, c * 128:(c + 1) * 128],
                                    identb[:, :])
                nc.tensor.transpose(ps_tr[:, 2 + c, :],
                                    gf[:, c * 128:(c + 1) * 128],
                                    identb[:, :])
            xh = spool.tile([128, 4, 128], BF16, name=f"xh_{j}_{t}", tag="xh")
            nc.vector.tensor_copy(xh[:, :, :], ps_tr[:, :, :])
            ps_rz = prz.tile([128, 2 * F], F32, name=f"psrz_{j}_{t}", tag="rz")
            mmi = 0
            for base, wt in ((0, wih[t]), (2, whh[t])):
                for c in range(2):
                    nc.tensor.matmul(ps_rz[:, :], xh[:, base + c, :],
                                     wt[:, c, 0:2 * F],
                                     start=(mmi == 0), stop=False)
                    mmi += 1
            nc.tensor.matmul(ps_rz[:, :], onesrow[:, :], brzrow[t][:, :],
                             start=False, stop=True)
            ps_in = pacc.tile([128, F], F32, name=f"psin_{j}_{t}", tag="acc")
            for c in range(2):
                nc.tensor.matmul(ps_in[:, :], xh[:, c, :],
                                 wih[t][:, c, 2 * F:3 * F],
                                 start=(c == 0), stop=False)
            nc.tensor.matmul(ps_in[:, :], onesrow[:, :], binrow[t][:, :],
                             start=False, stop=True)
            ps_hn = pacc.tile([128, F], F32, name=f"pshn_{j}_{t}", tag="acc")
            for c in range(2):
                nc.tensor.matmul(ps_hn[:, :], xh[:, 2 + c, :],
                                 whh[t][:, c, 2 * F:3 * F],
                                 start=(c == 0), stop=False)
            nc.tensor.matmul(ps_hn[:, :], onesrow[:, :], bhnrow[t][:, :],
                             start=False, stop=True)
            rza = spool.tile([128, 2 * F], BF16, name=f"rza_{j}_{t}", tag="rza")
            nc.scalar.activation(rza[:, :], ps_rz[:, :], ACT.Sigmoid)
            tmp = spool.tile([128, F], BF16, name=f"tmp_{j}_{t}", tag="tmp")
            nc.vector.tensor_tensor(tmp[:, :], ps_hn[:, :], rza[:, 0:F],
                                    op=AOP.mult)
            t2 = spool.tile([128, F], F32, name=f"t2_{j}_{t}", tag="t2")
            nc.vector.tensor_tensor(t2[:, :], ps_in[:, :], tmp[:, :], op=AOP.add)
            nn = spool.tile([128, F], BF16, name=f"nn_{j}_{t}", tag="nn")
            nc.scalar.activation(nn[:, :], t2[:, :], ACT.Tanh)
            hm = spool.tile([128, F], BF16, name=f"hm_{j}_{t}", tag="hm")
            nc.gpsimd.tensor_tensor(hm[:, :], gf[:, :], nn[:, :], op=AOP.subtract)
            hz = spool.tile([128, F], BF16, name=f"hz_{j}_{t}", tag="hz")
            nc.vector.tensor_tensor(hz[:, :], hm[:, :], rza[:, F:2 * F],
                                    op=AOP.mult)
            if t < T - 1:
                gf_new = spool.tile([128, F], BF16, name=f"gfn_{j}_{t}", tag="gf", bufs=8)
                nc.vector.tensor_tensor(gf_new[:, :], hz[:, :], nn[:, :],
                                        op=AOP.add)
                s["gf"] = gf_new
            else:
                gout = spool.tile([128, F], F32, name=f"gout_{j}", tag="gout")
                nc.vector.tensor_tensor(gout[:, :], hz[:, :], nn[:, :],
                                        op=AOP.add)
                nc.scalar.dma_start(out_d[j * 128:(j + 1) * 128, :], gout[:, :])
                del S[j]

        pairs = [tuple(j for j in (j0, j0 + 1) if j < NT)
                 for j0 in range(0, NT, 2)]
        # software pipeline: pair P+1's DMA issues at the start of pair P;
        # pair P+1's g0/w01 matmuls are emitted between P's two timesteps
        # to fill the PE bubble at the t0->t1 dependency chain.
        for j in pairs[0]:
            emit_dma(j)
        for j in pairs[0]:
            emit_g0(j)
        for j in pairs[0]:
            emit_w01(j)
        for p, pair in enumerate(pairs):
            nxt = pairs[p + 1] if p + 1 < len(pairs) else ()
            for j in nxt:
                emit_dma(j)
            for t in range(T):
                for j in pair:
                    emit_u(j, t)
                for j in pair:
                    emit_ubcv(j, t)
                for j in pair:
                    emit_echain(j, t)
                for j in pair:
                    emit_ds(j, t)
                for j in pair:
                    emit_stl(j, t)
                for j in pair:
                    emit_ctx(j, t)
                for j in pair:
                    emit_gru(j, t)
                if t == 0:
                    for j in nxt:
                        emit_g0(j)
                    for j in nxt:
                        emit_w01(j)
    nc.finalize()
    return nc, ctx


def _prep_core(node_feats_bf, seg, g_lo, g_hi, NT, NSUB):
    """Stage one core's node data in the four device layouts."""
    NN = NSUB * 128
    nfaug = np.zeros((128, NT, NSUB, F + 1), NP_BF16)
    nft = np.zeros((128, NT, 2, NN), NP_BF16)
    mn = np.zeros((128, NT, 128, NSUB), NP_BF16)
    mnt = np.zeros((128, NT, NSUB, 128), NP_BF16)
    eye = np.eye(128, dtype=NP_BF16)
    for j in range(NT):
        gt = g_lo + j * 128
        if gt >= g_hi:
            continue
        ge = min(gt + 128, g_hi)
        a = int(np.searchsorted(seg, gt, 'left'))
        b = int(np.searchsorted(seg, ge, 'left'))
        cnt = b - a
        assert cnt <= NN
        tmp = np.zeros((NN, F + 1), NP_BF16)
        tmp[:cnt, 0] = 1.0
        tmp[:cnt, 1:] = node_feats_bf[a:b]
        # node n -> subtile s=n//128, partition p=n%128
        nfaug[:, j] = tmp.reshape(NSUB, 128, F + 1).transpose(1, 0, 2)
        # nft[fp, j, c, n] = nf[a+n, c*128+fp]
        nft[:, j] = np.ascontiguousarray(
            tmp[:, 1:].T.reshape(2, 128, NN).transpose(1, 0, 2))
        grel = np.full(NN, -1, np.int64)
        grel[:cnt] = seg[a:b] - gt
        oh = eye[np.clip(grel, 0, 127)] * (grel >= 0)[:, None].astype(NP_BF16)
        oh = oh.reshape(NSUB, 128, 128)          # [s, p, g]
        mn[:, j] = oh.transpose(1, 2, 0)         # [p, g, s]
        mnt[:, j] = oh.transpose(2, 0, 1)        # [g, s, p]
    return nfaug, nft, mn, mnt


def kernel(node_feats, seg_ids, Wl, bl, Wp, bp, Wih, Whh, bih, bhh):
    node_feats = np.asarray(node_feats, np.float32)
    seg = np.asarray(seg_ids).astype(np.int64)
    Wl = np.asarray(Wl, np.float32)
    bl = np.asarray(bl, np.float32)
    Wp = np.asarray(Wp, np.float32)
    bp = np.asarray(bp, np.float32)
    Wih = np.asarray(Wih, np.float32)
    Whh = np.asarray(Whh, np.float32)
    bih = np.asarray(bih, np.float32)
    bhh = np.asarray(bhh, np.float32)
    V = node_feats.shape[0]
    G = 25000

    bounds_g = [0]
    for c in range(1, NCORES):
        bounds_g.append(int(seg[c * V // NCORES]))
    bounds_g.append(G)

    NT = max((bounds_g[c + 1] - bounds_g[c] + 127) // 128 for c in range(NCORES))
    maxnodes = 1
    for c in range(NCORES):
        for gt in range(bounds_g[c], bounds_g[c + 1], 128):
            ge = min(gt + 128, bounds_g[c + 1])
            a = np.searchsorted(seg, gt, 'left')
            b = np.searchsorted(seg, ge, 'left')
            maxnodes = max(maxnodes, int(b - a))
    NSUB = (maxnodes + 127) // 128

    nc, ctx = _build_program(NT, NSUB, [float(bl[t, 0]) for t in range(T)])

    shared = {
        "identb": np.eye(128, dtype=NP_BF16),
        "onesrow": np.ones((1, 128), NP_BF16),
    }
    wl2 = np.zeros((128, 2, T), np.float32)
    for t in range(T):
        for c in range(2):
            wl2[:, c, t] = Wl[t, 0, F + c * 128:F + (c + 1) * 128]
    shared["wl2"] = wl2.astype(NP_BF16)
    for t in range(T):
        shared[f"wlg{t}"] = np.broadcast_to(Wl[t, 0, :F], (128, F)).astype(NP_BF16)
        shared[f"wpt{t}"] = np.ascontiguousarray(
            Wp[t].T.reshape(2, 128, F).transpose(1, 0, 2)).astype(NP_BF16)
        shared[f"wih{t}"] = np.ascontiguousarray(
            Wih[t].T.reshape(2, 128, 3 * F).transpose(1, 0, 2)).astype(NP_BF16)
        shared[f"whh{t}"] = np.ascontiguousarray(
            Whh[t].T.reshape(2, 128, 3 * F).transpose(1, 0, 2)).astype(NP_BF16)
        shared[f"bprow{t}"] = bp[t][None, :].astype(NP_BF16)
        # elu's -1 shifted into the GRU input bias: x_gru = ctx+1 staged,
        # so bias_x -= rowsum(Wih)
        rs = Wih[t].sum(axis=1)
        shared[f"brzrow{t}"] = (bih[t, :2 * F] + bhh[t, :2 * F] - rs[:2 * F])[None, :].astype(NP_BF16)
        shared[f"binrow{t}"] = (bih[t, 2 * F:] - rs[2 * F:])[None, :].astype(NP_BF16)
        shared[f"bhnrow{t}"] = bhh[t, 2 * F:][None, :].astype(NP_BF16)

    node_feats_bf = node_feats.astype(NP_BF16)
    in_maps = []
    for c in range(NCORES):
        nfaug, nft, mn, mnt = _prep_core(
            node_feats_bf, seg, bounds_g[c], bounds_g[c + 1], NT, NSUB)
        m = dict(shared)
        m["nfaug"] = nfaug
        m["nft"] = nft
        m["mn"] = mn
        m["mnt"] = mnt
        in_maps.append(m)

    res = run_bass_kernel_spmd(nc, in_maps, core_ids=list(range(NCORES)))
    ctx.close()
    global LAST_RESULT
    LAST_RESULT = res

    out = np.zeros((G, F), np.float32)
    for c in range(NCORES):
        gc = bounds_g[c + 1] - bounds_g[c]
        out[bounds_g[c]:bounds_g[c + 1]] = res.results[c]["out"][:gc]
    return out
